# revision 27
# baseline (speedup 1.0000x reference)
"""GNN edge-scorer kernel for Trainium2 (8 NeuronCores, SPMD).

reference:
    edge_emb = concat(emb[src], emb[dst])          # [E, 128]
    h = relu(edge_emb @ W1 + b1)                   # [E, 64]
    logits = h @ W2 + b2                           # [E, 1]

Device algorithm (memory-bound gather regime; BF16=True path):
  Phase A (per core, replicated): AB[n] = [emb[n]@W1[:64]*|w2| + b1*|w2| |
    emb[n]@W1[64:]*|w2|] via PE matmul with K=65 (ones-row folds b1), cast
    to bf16.  AB is [150016, 128] bf16 in DRAM, row = 256 B.  |w2| is folded
    into the table (relu(k x) = k relu(x), k>=0) with features permuted so
    positive-sign w2 features occupy cols [0:npos].  Table writes put
    PAIR_M=4 consecutive rows on one partition -> 1 KB write descriptors.
  Phase B: per edge, dma_gather full 256 B rows of AB[src] and AB[dst]
    (int16 idx, elem_step=128); DVE/ACT: h = relu(srcrow[0:64]+dstrow[64:128]);
    logit = sum(h[0:npos]) - sum(h[npos:64]) + b2.  The gather is
    descriptor-rate-bound (~44-48 ns per 256 B descriptor per engine slot,
    measured; address locality does not matter), so everything else overlaps
    behind it.

  dma_gather int16 indices only span 32768 rows -> table split into 5 chunks;
  host sorts edges into 25 (src_chunk, dst_chunk) buckets and deals them
  round-robin over the 8 cores so every core has identical bucket sizes
  (single SPMD program).  Output unscrambled on host.
"""

import numpy as np

N_NODES = 150000
TRACE = False            # set by test harness to capture HW profile
_last_results = None     # BassKernelResults of the most recent run
_last_ctx = None         # (nc, in_maps) of the most recent run
_last_names = None
_last_s_b = None
D = 64
P = 128
N_CORES = 8
CHUNK = 32768                      # dma_gather int16 index range
NT = 1024                          # precompute nodes per tile
MAXG = 8192                        # max indices per dma_gather instruction
QUEUES = 4                         # SWDGE queues for gather desc-gen
SINGLE_PACKET = False              # dma_gather single_packet flag
SORT_SRC = False                   # secondary-sort edges by src within bucket
PAIR_M = 8                         # nodes per partition per AB-write descriptor (bf16)
SPLIT_GATHER = 2                   # split each dma_gather into this many queue-parallel parts


def _derived():
    n_chunks = (N_NODES + CHUNK - 1) // CHUNK
    # pad so every chunk (incl. the last) splits into full NT=1024 tiles:
    # PAIR_M=8 write grouping needs n % (PAIR_M*128) == 0 per tile
    r_pad = CHUNK * (n_chunks - 1) + NT * (
        (N_NODES - CHUNK * (n_chunks - 1) + NT - 1) // NT
    )
    return n_chunks, r_pad


def _round_up(x, m):
    return (x + m - 1) // m * m


BF16 = True


def _build_program(s_b, npos=D, loop_n=None, mode="full", queues=1, unroll2=False, b2_zero=False):
    """Build the SPMD Bass program.

    s_b: list of 25 per-bucket slot counts (slots of 128 edges, same on
    every core).  Returns (nc, names dict).

    BF16=True: AB table stored bf16 (row = 128 bf16 = 256B); gathers pull
    the full row for both endpoints and the compute uses the A half of the
    src row and the B half of the dst row.  Precompute writes pair nodes
    (2p, 2p+1) per partition so each write descriptor is 512B.

    |w2| is folded into the table columns on the host (relu(k*x) = k*relu(x)
    for k>=0) and features are permuted so cols [0:npos] carry positive w2
    sign and [npos:D] negative: logit = sum(relu[0:npos]) - sum(relu[npos:D])
    + b2, eliminating the per-edge w2 multiply.
    """
    import concourse.bacc as bacc
    import concourse.mybir as mybir
    import concourse.tile as tile

    N_CHUNKS, R_PAD = _derived()
    tot_slots = int(sum(s_b))
    tot_idx = tot_slots * P
    dt = mybir.dt.bfloat16 if BF16 else mybir.dt.float32

    nc = bacc.Bacc(None, target_bir_lowering=False, debug=False, num_swdge_queues=queues)
    with tile.TileContext(nc) as tc:
        with tc.tile_pool(name="dram", bufs=1, space="DRAM") as dram:
            embT_t = dram.tile([D + 1, R_PAD], dt, kind="ExternalInput")
            w1cat_t = dram.tile([D + 1, 2 * D], dt, kind="ExternalInput")
            w2rep_t = dram.tile([P, D], dt, kind="ExternalInput")
            b2b_t = dram.tile([P, 1], mybir.dt.float32, kind="ExternalInput")
            isrc_t = dram.tile([P, tot_idx // 16], mybir.dt.int16, kind="ExternalInput")
            idst_t = dram.tile([P, tot_idx // 16], mybir.dt.int16, kind="ExternalInput")
            out_t = dram.tile([P, tot_slots], mybir.dt.float32, kind="ExternalOutput")

            # AB table, one DRAM tile per 32768-row chunk so Tile can
            # pipeline gathers of chunk c behind the writes of chunk c.
            ab_rows = [CHUNK] * (N_CHUNKS - 1) + [R_PAD - CHUNK * (N_CHUNKS - 1)]
            ab_sets = [[
                dram.tile([r, 2 * D], dt, name=f"ab{c}_{u}")
                for c, r in enumerate(ab_rows)
            ] for u in range(2 if unroll2 else 1)]

            with (
                tc.tile_pool(name="consts", bufs=1) as consts,
                tc.tile_pool(name="pre_et", bufs=3) as pre_et,
                tc.tile_pool(name="pre_ab", bufs=3) as pre_ab,
                tc.tile_pool(name="ps_ab", bufs=2, space="PSUM") as ps_ab,
                tc.tile_pool(name="idx", bufs=4) as idxp,
                tc.tile_pool(name="ga", bufs=4 if BF16 else 3) as gap,
                tc.tile_pool(name="h", bufs=4) as hp,
                tc.tile_pool(name="lg", bufs=4) as lgp,
            ):
                w1cat_sb = consts.tile([D + 1, 2 * D], dt)
                nc.sync.dma_start(w1cat_sb[:], w1cat_t[:])
                b2b_sb = consts.tile([P, 1], mybir.dt.float32)
                nc.sync.dma_start(b2b_sb[:], b2b_t[:])
                # BF16: w2rep_t carries the +-1 sign vector (|w2| is folded
                # into the table); f32: it carries w2 itself.  Either way,
                # replicate across MAXG//128 slots -> [P, (MAXG//P)*D].
                w2in_sb = consts.tile([P, D], dt, tag="w2in")
                nc.sync.dma_start(w2in_sb[:], w2rep_t[:])
                w2rep_sb = consts.tile([P, (MAXG // P) * D], dt, tag="w2rep")
                for j in range(MAXG // P):
                    nc.vector.tensor_copy(w2rep_sb[:, j * D:(j + 1) * D], w2in_sb[:])

                # ---- interleaved emission: precompute chunks + buckets ----
                def precompute_chunk(c, tabs):
                    base = c * CHUNK
                    rows = ab_rows[c]
                    for nt0 in range(0, rows, NT):
                        n = min(NT, rows - nt0)
                        et = pre_et.tile([D + 1, NT], dt, tag="et")
                        nc.sync.dma_start(
                            et[:, :n], embT_t[:, base + nt0: base + nt0 + n]
                        )
                        ab_sb = pre_ab.tile([P, NT // P * 2 * D], dt, tag="absb")
                        assert n % (4 * P) == 0
                        for k4 in range(n // (4 * P)):
                            # 4 matmuls fill one PSUM bank -> one wide copy
                            ab_ps = ps_ab.tile([P, 512], mybir.dt.float32, space="PSUM")
                            for m in range(4):
                                k = 4 * k4 + m
                                nc.tensor.matmul(
                                    ab_ps[:, m * 2 * D:(m + 1) * 2 * D],
                                    lhsT=et[:, k * P:(k + 1) * P],
                                    rhs=w1cat_sb[:],
                                    start=True, stop=True,
                                )
                            nc.scalar.activation(
                                ab_sb[:, k4 * 512:(k4 + 1) * 512],
                                ab_ps[:],
                                mybir.ActivationFunctionType.Copy,
                            )
                        if BF16:
                            # host permuted emb columns so matmul k covers
                            # device nodes {nt0 + (k//M)*(M*128) + M*p + k%M}:
                            # rows M*p..M*p+M-1 of each group sit on partition
                            # p -> M*256B contiguous write descriptors.
                            M = PAIR_M
                            assert n % (M * P) == 0
                            dst = tabs[c][nt0:nt0 + n, :].rearrange(
                                "(g p m) f -> p g m f", p=P, m=M
                            )
                            src_ap = ab_sb[:, :n // P * 2 * D].rearrange(
                                "p (g m f) -> p g m f", m=M, f=2 * D
                            )
                        else:
                            # AB rows nt0+k*128+p , feat f  <- ab_sb[p, k*128+f]
                            dst = tabs[c][nt0:nt0 + n, :].rearrange(
                                "(k p) f -> p k f", p=P
                            )
                            src_ap = ab_sb[:, :n // P * 2 * D].rearrange(
                                "p (k f) -> p k f", f=2 * D
                            )
                        nc.sync.dma_start(dst, src_ap)

                # bucket schedule: emit bucket (cs,cd) after chunks cs, cd
                emitted_chunks = set()
                idx_off = 0   # in int16 columns (tot_idx//16 total)
                out_off = 0   # in slots

                gq = [0]

                def emit_bucket(b, tabs):
                    nonlocal idx_off, out_off
                    cs, cd = b // N_CHUNKS, b % N_CHUNKS
                    slots = s_b[b]
                    done = 0
                    while done < slots:
                        g = min(MAXG // P, slots - done)      # slots this gather
                        n_idx = g * P
                        isrc = idxp.tile([P, MAXG // 16], mybir.dt.int16, tag="isrc")
                        idst = idxp.tile([P, MAXG // 16], mybir.dt.int16, tag="idst")
                        nc.sync.dma_start(
                            isrc[:, :n_idx // 16],
                            isrc_t[:, idx_off: idx_off + n_idx // 16],
                        )
                        nc.sync.dma_start(
                            idst[:, :n_idx // 16],
                            idst_t[:, idx_off: idx_off + n_idx // 16],
                        )
                        if BF16:
                            # full 256B rows; use A half of src, B half of dst
                            ga = gap.tile([P, MAXG // P, 2 * D], dt, tag="ga")
                            gb = gap.tile([P, MAXG // P, 2 * D], dt, tag="gb")
                            ga_h = ga[:, :g, 0:D]
                            gb_h = gb[:, :g, D:2 * D]
                            gath = [
                                (ga, tabs[cs][:, :], isrc),
                                (gb, tabs[cd][:, :], idst),
                            ]
                            esz, estep = 2 * D, 2 * D
                        else:
                            ga = gap.tile([P, MAXG // P, D], mybir.dt.float32, tag="ga")
                            gb = gap.tile([P, MAXG // P, D], mybir.dt.float32, tag="gb")
                            ga_h = ga[:, :g, :]
                            gb_h = gb[:, :g, :]
                            gath = [
                                (ga, tabs[cs][:, 0:D], isrc),
                                (gb, tabs[cd][:, D:2 * D], idst),
                            ]
                            esz, estep = D, 2 * D
                        if mode not in ("B_dve",):
                            ns = SPLIT_GATHER if (g * P) % (SPLIT_GATHER * P) == 0 else 1
                            npart = n_idx // ns
                            for gt, src_ap, it in gath:
                                for si in range(ns):
                                    nc.gpsimd.dma_gather(
                                        gt[:, si * (g // ns):(si + 1) * (g // ns), :],
                                        src_ap,
                                        it[:, si * npart // 16:(si + 1) * npart // 16],
                                        npart, npart, esz, elem_step=estep,
                                        single_packet=SINGLE_PACKET, queue_num=gq[0],
                                    )
                                    gq[0] = (gq[0] + 1) % queues
                        # h = relu(ga + gb); logit = sum(h*w2) + b2
                        if mode in ("B_gather", "B_gp"):
                            idx_off += n_idx // 16
                            out_off += g
                            done += g
                            continue
                        lg = lgp.tile([P, MAXG // P], mybir.dt.float32, tag="lg")
                        if BF16:
                            # h in a separate CONTIGUOUS tile (unit-stride DVE,
                            # ga/gb release right after the add), then ONE
                            # fused DVE op does relu AND the +-1 sign multiply
                            # (|w2| pre-folded into the table), then a single
                            # full-width reduce.
                            h = hp.tile([P, MAXG // P, D], dt, tag="h")
                            ga_h = h[:, :g, :]
                            nc.vector.tensor_add(ga_h, ga[:, :g, 0:D], gb_h)
                            nc.vector.scalar_tensor_tensor(
                                ga_h, ga_h, 0.0,
                                w2rep_sb[:, :g * D].rearrange("p (s d) -> p s d", d=D),
                                op0=mybir.AluOpType.max,
                                op1=mybir.AluOpType.mult,
                            )
                            nc.vector.tensor_reduce(
                                lg[:, :g], ga_h,
                                axis=mybir.AxisListType.X,
                                op=mybir.AluOpType.add,
                            )
                        else:
                            nc.vector.tensor_add(ga_h, ga_h, gb_h)
                            nc.scalar.activation(
                                ga_h, ga_h, mybir.ActivationFunctionType.Relu,
                            )
                            nc.vector.tensor_mul(
                                ga_h, ga_h,
                                w2rep_sb[:, :g * D].rearrange("p (s d) -> p s d", d=D),
                            )
                            nc.vector.tensor_reduce(
                                lg[:, :g], ga_h,
                                axis=mybir.AxisListType.X, op=mybir.AluOpType.add,
                            )
                        if not b2_zero:
                            nc.vector.tensor_scalar_add(
                                lg[:, :g], lg[:, :g], b2b_sb[:, :1]
                            )
                        nc.sync.dma_start(
                            out_t[:, out_off: out_off + g], lg[:, :g]
                        )
                        idx_off += n_idx // 16
                        out_off += g
                        done += g

                # schedule: smallest chunk first (shorter pre-gather head),
                # then after step i, all buckets whose chunks are ready.
                # chunk_order/pos MUST match prepare()'s emit_order.
                chunk_order = sorted(range(N_CHUNKS), key=lambda c: (ab_rows[c], c))
                cpos = {c: i for i, c in enumerate(chunk_order)}

                def emit_all(tabs):
                    nonlocal idx_off, out_off
                    idx_off = 0
                    out_off = 0
                    for i, c in enumerate(chunk_order):
                        precompute_chunk(c, tabs)
                        emitted_chunks.add(c)
                        for b in range(N_CHUNKS * N_CHUNKS):
                            cs, cd = b // N_CHUNKS, b % N_CHUNKS
                            if max(cpos[cs], cpos[cd]) == i and s_b[b] > 0:
                                emit_bucket(b, tabs)

                def emit_buckets_only(tabs):
                    nonlocal idx_off, out_off
                    idx_off = 0
                    out_off = 0
                    for i, c in enumerate(chunk_order):
                        for b in range(N_CHUNKS * N_CHUNKS):
                            cs, cd = b // N_CHUNKS, b % N_CHUNKS
                            if max(cpos[cs], cpos[cd]) == i and s_b[b] > 0:
                                emit_bucket(b, tabs)

                if loop_n is None:
                    emit_all(ab_sets[0])
                elif mode in ("full", "B_gp"):
                    with tc.For_i(0, loop_n, 1):
                        for abs_u in ab_sets:
                            emit_all(abs_u)
                else:
                    for c in range(N_CHUNKS):
                        precompute_chunk(c, ab_sets[0])
                    with tc.For_i(0, loop_n, 1):
                        for abs_u in ab_sets:
                            emit_buckets_only(abs_u)

    nc.compile()
    names = dict(
        embT=embT_t.name, w1cat=w1cat_t.name, w2rep=w2rep_t.name,
        b2b=b2b_t.name, isrc=isrc_t.name, idst=idst_t.name, out=out_t.name,
    )
    return nc, names, tot_slots


def prepare(nodes_emb, src, dst, W1, b1, W2, b2):
    """Host prep: bucket sort, index packing, input arrays. Returns a dict."""
    nodes_emb = np.ascontiguousarray(np.asarray(nodes_emb, dtype=np.float32))
    src = np.asarray(src).astype(np.int64)
    dst = np.asarray(dst).astype(np.int64)
    W1 = np.asarray(W1, dtype=np.float32)
    b1 = np.asarray(b1, dtype=np.float32).reshape(-1)
    W2 = np.asarray(W2, dtype=np.float32)
    b2 = np.asarray(b2, dtype=np.float32).reshape(-1)
    E = src.shape[0]
    N_CHUNKS, R_PAD = _derived()

    # ---- host prep -------------------------------------------------------
    # embT65 [65, R_PAD]: emb^T padded with zeros, plus a ones row (bias)
    embT = np.zeros((D + 1, R_PAD), dtype=np.float32)
    embT[:D, :N_NODES] = nodes_emb.T
    embT[D, :] = 1.0
    # w1cat65 [65, 128]: cols 0:64 -> A-half (W1[:64] with b1), 64:128 -> B-half
    w1cat = np.zeros((D + 1, 2 * D), dtype=np.float32)
    w1cat[:D, :D] = W1[:D]
    w1cat[:D, D:] = W1[D:]
    w1cat[D, :D] = b1            # bias folded into A-half
    w2rep = np.tile(W2.reshape(1, D), (P, 1)).astype(np.float32)
    b2b = np.full((P, 1), b2[0], dtype=np.float32)
    npos = D
    if BF16:
        import ml_dtypes
        # fold |w2| into the table columns; permute features so +sign w2
        # features come first (cols [0:npos]) and -sign after.
        w2v = W2.reshape(D)
        fperm = np.argsort(w2v < 0, kind="stable")     # positives first
        npos = int((w2v >= 0).sum())
        scale = np.abs(w2v[fperm])                     # >= 0
        w1cat = w1cat[:, np.concatenate([fperm, D + fperm])] * np.concatenate([scale, scale])
        # device matmul k covers table rows (k//M)*(M*128) + M*p + (k%M);
        # permute emb columns so table row r holds node r's data.
        M = PAIR_M
        j = np.arange(R_PAD)
        k, p_ = j // P, j % P
        perm = (k // M) * (M * P) + M * p_ + (k % M)
        embT = embT[:, perm].astype(ml_dtypes.bfloat16)
        w1cat = w1cat.astype(ml_dtypes.bfloat16)
        sgn = np.where(w2v[fperm] >= 0, 1.0, -1.0).astype(np.float32)
        w2rep = np.tile(sgn.reshape(1, D), (P, 1)).astype(ml_dtypes.bfloat16)

    # ---- bucket sort + deal over cores ----------------------------------
    cs = src // CHUNK
    cd = dst // CHUNK
    bucket = (cs * N_CHUNKS + cd).astype(np.int64)
    if SORT_SRC:
        # ascending src rows inside each bucket: gather descriptors hit
        # increasing HBM addresses (row-buffer friendly)
        order = np.lexsort((src, bucket))
    else:
        order = np.argsort(bucket, kind="stable")      # edge ids, bucket-major
    bcounts = np.bincount(bucket, minlength=N_CHUNKS * N_CHUNKS)
    # per-core-per-bucket count (round-robin deal), padded to 128
    m_bc = -(-bcounts // N_CORES)                      # ceil
    s_b = [int(_round_up(m, P) // P) if m > 0 else 0 for m in m_bc]
    tot_slots = int(sum(s_b))
    tot_idx = tot_slots * P

    src_s = (src[order] % CHUNK).astype(np.int16)
    dst_s = (dst[order] % CHUNK).astype(np.int16)

    # host index bookkeeping: for each sorted position, compute its
    # (core, flat device stream index)
    core_of = np.empty(E, dtype=np.int64)
    stream_of = np.empty(E, dtype=np.int64)
    bstart = np.concatenate([[0], np.cumsum(bcounts)])
    # device consumes buckets in emit order: smallest chunk precomputed
    # first; bucket (cs, cd) emitted at the step both chunks are ready.
    # MUST match _build_program's chunk_order/cpos.
    ab_rows_h = [CHUNK] * (N_CHUNKS - 1) + [R_PAD - CHUNK * (N_CHUNKS - 1)]
    chunk_order = sorted(range(N_CHUNKS), key=lambda c: (ab_rows_h[c], c))
    cpos = {c: i for i, c in enumerate(chunk_order)}
    emit_order = [
        b for i in range(N_CHUNKS)
        for b in range(N_CHUNKS * N_CHUNKS)
        if max(cpos[b // N_CHUNKS], cpos[b % N_CHUNKS]) == i
    ]
    slot_off = np.zeros(N_CHUNKS * N_CHUNKS, dtype=np.int64)
    acc = 0
    for b in emit_order:
        slot_off[b] = acc
        acc += s_b[b]
    for b in range(N_CHUNKS * N_CHUNKS):
        nb = bcounts[b]
        if nb == 0:
            continue
        pos = np.arange(nb)
        core_of[bstart[b]: bstart[b + 1]] = pos % N_CORES
        stream_of[bstart[b]: bstart[b + 1]] = slot_off[b] * P + pos // N_CORES

    isrc_all = np.zeros((N_CORES, tot_idx), dtype=np.int16)
    idst_all = np.zeros((N_CORES, tot_idx), dtype=np.int16)
    for c in range(N_CORES):
        m = core_of == c
        isrc_all[c, stream_of[m]] = src_s[m]
        idst_all[c, stream_of[m]] = dst_s[m]

    def wrap16(a):
        # stream index i -> [i % 16, i // 16], replicated to 128 partitions
        w = a.reshape(-1, 16).T                        # [16, tot/16]
        return np.tile(w, (8, 1)).copy()

    return dict(
        E=E, s_b=s_b, npos=npos, b2_zero=bool(b2[0] == 0.0),
        core_of=core_of, stream_of=stream_of,
        order=order, embT=embT, w1cat=w1cat, w2rep=w2rep, b2b=b2b,
        isrc=[wrap16(isrc_all[c]) for c in range(N_CORES)],
        idst=[wrap16(idst_all[c]) for c in range(N_CORES)],
    )


def make_in_maps(prep, names):
    return [
        {
            names["embT"]: prep["embT"],
            names["w1cat"]: prep["w1cat"],
            names["w2rep"]: prep["w2rep"],
            names["b2b"]: prep["b2b"],
            names["isrc"]: prep["isrc"][c],
            names["idst"]: prep["idst"][c],
        }
        for c in range(N_CORES)
    ]


def run_prep(prep, loop_n=None, unroll2=False):
    """Build the program (optionally with a repeat loop) and run it once."""
    from concourse.bass_utils import run_bass_kernel_spmd

    nc, names, tot_slots = _build_program(
        prep["s_b"], npos=prep.get("npos", D), loop_n=loop_n, queues=QUEUES,
        unroll2=unroll2, b2_zero=prep.get("b2_zero", False),
    )
    in_maps = make_in_maps(prep, names)
    res = run_bass_kernel_spmd(
        nc, in_maps, core_ids=list(range(N_CORES)), trace=False,
    )
    global _last_results, _last_ctx, _last_names, _last_s_b
    _last_results = res
    _last_ctx = (nc, in_maps)
    _last_names = names
    _last_s_b = prep["s_b"]
    return res, names


def unscramble(prep, res, names):
    # device out [128, tot_slots]: stream index i -> out[i % 128, i // 128]
    E = prep["E"]
    core_of, stream_of, order = prep["core_of"], prep["stream_of"], prep["order"]
    logits_sorted = np.empty(E, dtype=np.float32)
    for c in range(N_CORES):
        o = res.results[c][names["out"]]               # [128, tot_slots]
        m = core_of == c
        si = stream_of[m]
        logits_sorted[np.flatnonzero(m)] = o[si % P, si // P]
    out = np.empty(E, dtype=np.float32)
    out[order] = logits_sorted
    return out.reshape(E, 1)


def kernel(nodes_emb, src, dst, W1, b1, W2, b2):
    prep = prepare(nodes_emb, src, dst, W1, b1, W2, b2)
    res, names = run_prep(prep, loop_n=None)
    return unscramble(prep, res, names)


def measure_hw(prep, r1=8, r2=64, n_iters=10, unroll2=False):
    """Differential HW timing: the program body repeated r inside one NEFF
    dispatch; per-execution time = (wall(r2) - wall(r1)) / (execs2 - execs1).

    The axon/PJRT dispatch overhead (~80 ms, validated with a trivial
    kernel) cancels in the difference.  With unroll2, each loop iteration
    holds TWO complete kernel executions on alternating DRAM table sets
    (removes the artificial write-after-read serialization at the loop
    back-edge that a single-shot run does not have).  Also verifies the
    loop programs produce the same outputs as the single-shot program.
    """
    mult = 2 if unroll2 else 1
    walls = {}
    outs = {}
    for r in (r1, r2):
        res, names = run_prep(prep, loop_n=r, unroll2=unroll2)
        outs[r] = unscramble(prep, res, names)
        ts = bench(n_iters=n_iters)
        walls[r] = min(ts)
        print(f"  loop_n={r} (x{mult}): wall min {walls[r]*1e3:.2f} ms "
              f"(iters: {[f'{t*1e3:.1f}' for t in ts]})")
    per_iter = (walls[r2] - walls[r1]) / ((r2 - r1) * mult)
    return per_iter, outs[r1], outs[r2]


def bench(n_iters=16, n_warmup=3):
    """Re-execute the last-compiled SPMD program on device-resident inputs.

    Returns list of per-iteration wall seconds (device exec + dispatch).
    """
    import time
    import jax
    import numpy as np
    from jax.sharding import Mesh, PartitionSpec
    from jax.experimental.shard_map import shard_map
    import concourse.mybir as mybir
    from concourse import bass2jax

    nc, in_maps = _last_ctx
    n_cores = len(in_maps)
    partition_name = nc.partition_id_tensor.name if nc.partition_id_tensor else None

    in_names, out_names, out_avals, zero_outs = [], [], [], []
    for alloc in nc.m.functions[0].allocations:
        if not isinstance(alloc, mybir.MemoryLocationSet):
            continue
        name = alloc.memorylocations[0].name
        if alloc.kind == "ExternalInput":
            if name != partition_name:
                in_names.append(name)
        elif alloc.kind == "ExternalOutput":
            shape = tuple(alloc.tensor_shape)
            dtype = mybir.dt.np(alloc.dtype)
            out_names.append(name)
            out_avals.append(jax.core.ShapedArray(shape, dtype))
            zero_outs.append(np.zeros(shape, dtype))
    n_params = len(in_names)
    n_outs = len(out_avals)
    in_names_all = in_names + out_names
    if partition_name is not None:
        in_names_all = in_names_all + [partition_name]

    def _body(*args):
        operands = list(args)
        if partition_name is not None:
            operands.append(bass2jax.partition_id_tensor())
        outs = bass2jax._bass_exec_p.bind(
            *operands,
            out_avals=tuple(out_avals),
            in_names=tuple(in_names_all),
            out_names=tuple(out_names),
            lowering_input_output_aliases=(),
            sim_require_finite=True,
            sim_require_nnan=True,
            nc=nc,
        )
        return tuple(outs)

    devices = jax.devices()[:n_cores]
    mesh = Mesh(np.asarray(devices), ("core",))
    in_specs = (PartitionSpec("core"),) * (n_params + n_outs)
    out_specs = (PartitionSpec("core"),) * n_outs
    donate = tuple(range(n_params, n_params + n_outs))
    sharded = jax.jit(
        shard_map(_body, mesh=mesh, in_specs=in_specs, out_specs=out_specs,
                  check_rep=False),
        donate_argnums=donate, keep_unused=True,
    )
    sharding = jax.sharding.NamedSharding(mesh, PartitionSpec("core"))
    concat_in = [
        jax.device_put(
            np.concatenate([np.asarray(in_maps[c][name]) for c in range(n_cores)], axis=0),
            sharding,
        )
        for name in in_names
    ]
    jax.block_until_ready(concat_in)
    n_total = n_warmup + n_iters
    zero_sets = [
        [
            jax.device_put(
                np.zeros((n_cores * z.shape[0], *z.shape[1:]), z.dtype), sharding
            )
            for z in zero_outs
        ]
        for _ in range(n_total)
    ]
    jax.block_until_ready(zero_sets)

    times = []
    for i in range(n_total):
        t0 = time.perf_counter()
        out = sharded(*concat_in, *zero_sets[i])
        jax.block_until_ready(out)
        times.append(time.perf_counter() - t0)
        del out
    return times[n_warmup:]



# revision 28
# speedup vs baseline: 1.0532x; 1.0532x over previous
"""GNN edge-scorer kernel for Trainium2 (8 NeuronCores, SPMD).

reference:
    edge_emb = concat(emb[src], emb[dst])          # [E, 128]
    h = relu(edge_emb @ W1 + b1)                   # [E, 64]
    logits = h @ W2 + b2                           # [E, 1]

Device algorithm (memory-bound gather regime; BF16=True path):
  Phase A (per core, replicated): AB[n] = [emb[n]@W1[:64]*|w2| + b1*|w2| |
    emb[n]@W1[64:]*|w2|] via PE matmul with K=65 (ones-row folds b1), cast
    to bf16.  AB is [150016, 128] bf16 in DRAM, row = 256 B.  |w2| is folded
    into the table (relu(k x) = k relu(x), k>=0) with features permuted so
    positive-sign w2 features occupy cols [0:npos].  Table writes put
    PAIR_M=4 consecutive rows on one partition -> 1 KB write descriptors.
  Phase B: per edge, dma_gather full 256 B rows of AB[src] and AB[dst]
    (int16 idx, elem_step=128); DVE/ACT: h = relu(srcrow[0:64]+dstrow[64:128]);
    logit = sum(h[0:npos]) - sum(h[npos:64]) + b2.  The gather is
    descriptor-rate-bound (~44-48 ns per 256 B descriptor per engine slot,
    measured; address locality does not matter), so everything else overlaps
    behind it.

  dma_gather int16 indices only span 32768 rows -> table split into 5 chunks;
  host sorts edges into 25 (src_chunk, dst_chunk) buckets and deals them
  round-robin over the 8 cores so every core has identical bucket sizes
  (single SPMD program).  Output unscrambled on host.
"""

import numpy as np

N_NODES = 150000
TRACE = False            # set by test harness to capture HW profile
_last_results = None     # BassKernelResults of the most recent run
_last_ctx = None         # (nc, in_maps) of the most recent run
_last_names = None
_last_s_b = None
D = 64
P = 128
N_CORES = 8
CHUNK = 32768                      # dma_gather int16 index range
NT = 1024                          # precompute nodes per tile
MAXG = 8192                        # max indices per dma_gather instruction
QUEUES = 4                         # SWDGE queues for gather desc-gen
SINGLE_PACKET = False              # dma_gather single_packet flag
SORT_SRC = False                   # secondary-sort edges by src within bucket
PAIR_M = 8                         # nodes per partition per AB-write descriptor (bf16)
SPLIT_GATHER = 2                   # split each dma_gather into this many queue-parallel parts


def _derived():
    n_chunks = (N_NODES + CHUNK - 1) // CHUNK
    # pad so every chunk (incl. the last) splits into full NT=1024 tiles:
    # PAIR_M=8 write grouping needs n % (PAIR_M*128) == 0 per tile
    r_pad = CHUNK * (n_chunks - 1) + NT * (
        (N_NODES - CHUNK * (n_chunks - 1) + NT - 1) // NT
    )
    return n_chunks, r_pad


def _round_up(x, m):
    return (x + m - 1) // m * m


BF16 = True


def _build_program(s_b, npos=D, loop_n=None, mode="full", queues=1, unroll2=False):
    """Build the SPMD Bass program.

    s_b: list of 25 per-bucket slot counts (slots of 128 edges, same on
    every core).  Returns (nc, names dict).

    BF16=True: AB table stored bf16 (row = 128 bf16 = 256B); gathers pull
    the full row for both endpoints and the compute uses the A half of the
    src row and the B half of the dst row.  Precompute writes pair nodes
    (2p, 2p+1) per partition so each write descriptor is 512B.

    |w2| is folded into the table columns on the host (relu(k*x) = k*relu(x)
    for k>=0) and features are permuted so cols [0:npos] carry positive w2
    sign and [npos:D] negative: logit = sum(relu[0:npos]) - sum(relu[npos:D])
    + b2, eliminating the per-edge w2 multiply.
    """
    import concourse.bacc as bacc
    import concourse.mybir as mybir
    import concourse.tile as tile

    N_CHUNKS, R_PAD = _derived()
    tot_slots = int(sum(s_b))
    tot_idx = tot_slots * P
    dt = mybir.dt.bfloat16 if BF16 else mybir.dt.float32

    nc = bacc.Bacc(None, target_bir_lowering=False, debug=False, num_swdge_queues=queues)
    with tile.TileContext(nc) as tc:
        with tc.tile_pool(name="dram", bufs=1, space="DRAM") as dram:
            embT_t = dram.tile([D + 1, R_PAD], dt, kind="ExternalInput")
            w1cat_t = dram.tile([D + 1, 2 * D], dt, kind="ExternalInput")
            w2rep_t = dram.tile([P, D], dt, kind="ExternalInput")
            b2b_t = dram.tile([P, 1], mybir.dt.float32, kind="ExternalInput")
            isrc_t = dram.tile([P, tot_idx // 16], mybir.dt.int16, kind="ExternalInput")
            idst_t = dram.tile([P, tot_idx // 16], mybir.dt.int16, kind="ExternalInput")
            out_t = dram.tile([P, tot_slots], mybir.dt.float32, kind="ExternalOutput")

            # AB table, one DRAM tile per 32768-row chunk so Tile can
            # pipeline gathers of chunk c behind the writes of chunk c.
            ab_rows = [CHUNK] * (N_CHUNKS - 1) + [R_PAD - CHUNK * (N_CHUNKS - 1)]
            ab_sets = [[
                dram.tile([r, 2 * D], dt, name=f"ab{c}_{u}")
                for c, r in enumerate(ab_rows)
            ] for u in range(2 if unroll2 else 1)]

            with (
                tc.tile_pool(name="consts", bufs=1) as consts,
                tc.tile_pool(name="pre_et", bufs=3) as pre_et,
                tc.tile_pool(name="pre_ab", bufs=3) as pre_ab,
                tc.tile_pool(name="ps_ab", bufs=2, space="PSUM") as ps_ab,
                tc.tile_pool(name="idx", bufs=4) as idxp,
                tc.tile_pool(name="ga", bufs=4 if BF16 else 3) as gap,
                tc.tile_pool(name="h", bufs=4) as hp,
                tc.tile_pool(name="lg", bufs=4) as lgp,
            ):
                w1cat_sb = consts.tile([D + 1, 2 * D], dt)
                nc.sync.dma_start(w1cat_sb[:], w1cat_t[:])
                b2b_sb = consts.tile([P, 1], mybir.dt.float32)
                nc.sync.dma_start(b2b_sb[:], b2b_t[:])
                # BF16: w2rep_t carries the +-1 sign vector (|w2| is folded
                # into the table); f32: it carries w2 itself.  Either way,
                # replicate across MAXG//128 slots -> [P, (MAXG//P)*D].
                w2in_sb = consts.tile([P, D], dt, tag="w2in")
                nc.sync.dma_start(w2in_sb[:], w2rep_t[:])
                w2rep_sb = consts.tile([P, (MAXG // P) * D], dt, tag="w2rep")
                for j in range(MAXG // P):
                    nc.vector.tensor_copy(w2rep_sb[:, j * D:(j + 1) * D], w2in_sb[:])

                # ---- interleaved emission: precompute chunks + buckets ----
                def precompute_chunk(c, tabs):
                    base = c * CHUNK
                    rows = ab_rows[c]
                    for nt0 in range(0, rows, NT):
                        n = min(NT, rows - nt0)
                        et = pre_et.tile([D + 1, NT], dt, tag="et")
                        nc.sync.dma_start(
                            et[:, :n], embT_t[:, base + nt0: base + nt0 + n]
                        )
                        ab_sb = pre_ab.tile([P, NT // P * 2 * D], dt, tag="absb")
                        assert n % (4 * P) == 0
                        for k4 in range(n // (4 * P)):
                            # 4 matmuls fill one PSUM bank -> one wide copy
                            ab_ps = ps_ab.tile([P, 512], mybir.dt.float32, space="PSUM")
                            for m in range(4):
                                k = 4 * k4 + m
                                nc.tensor.matmul(
                                    ab_ps[:, m * 2 * D:(m + 1) * 2 * D],
                                    lhsT=et[:, k * P:(k + 1) * P],
                                    rhs=w1cat_sb[:],
                                    start=True, stop=True,
                                )
                            nc.scalar.activation(
                                ab_sb[:, k4 * 512:(k4 + 1) * 512],
                                ab_ps[:],
                                mybir.ActivationFunctionType.Copy,
                            )
                        if BF16:
                            # host permuted emb columns so matmul k covers
                            # device nodes {nt0 + (k//M)*(M*128) + M*p + k%M}:
                            # rows M*p..M*p+M-1 of each group sit on partition
                            # p -> M*256B contiguous write descriptors.
                            M = PAIR_M
                            assert n % (M * P) == 0
                            dst = tabs[c][nt0:nt0 + n, :].rearrange(
                                "(g p m) f -> p g m f", p=P, m=M
                            )
                            src_ap = ab_sb[:, :n // P * 2 * D].rearrange(
                                "p (g m f) -> p g m f", m=M, f=2 * D
                            )
                        else:
                            # AB rows nt0+k*128+p , feat f  <- ab_sb[p, k*128+f]
                            dst = tabs[c][nt0:nt0 + n, :].rearrange(
                                "(k p) f -> p k f", p=P
                            )
                            src_ap = ab_sb[:, :n // P * 2 * D].rearrange(
                                "p (k f) -> p k f", f=2 * D
                            )
                        nc.sync.dma_start(dst, src_ap)

                # bucket schedule: emit bucket (cs,cd) after chunks cs, cd
                emitted_chunks = set()
                idx_off = 0   # in int16 columns (tot_idx//16 total)
                out_off = 0   # in slots

                gq = [0]

                def emit_bucket(b, tabs):
                    nonlocal idx_off, out_off
                    cs, cd = b // N_CHUNKS, b % N_CHUNKS
                    slots = s_b[b]
                    done = 0
                    while done < slots:
                        g = min(MAXG // P, slots - done)      # slots this gather
                        n_idx = g * P
                        isrc = idxp.tile([P, MAXG // 16], mybir.dt.int16, tag="isrc")
                        idst = idxp.tile([P, MAXG // 16], mybir.dt.int16, tag="idst")
                        nc.sync.dma_start(
                            isrc[:, :n_idx // 16],
                            isrc_t[:, idx_off: idx_off + n_idx // 16],
                        )
                        nc.sync.dma_start(
                            idst[:, :n_idx // 16],
                            idst_t[:, idx_off: idx_off + n_idx // 16],
                        )
                        if BF16:
                            # full 256B rows; use A half of src, B half of dst
                            ga = gap.tile([P, MAXG // P, 2 * D], dt, tag="ga")
                            gb = gap.tile([P, MAXG // P, 2 * D], dt, tag="gb")
                            ga_h = ga[:, :g, 0:D]
                            gb_h = gb[:, :g, D:2 * D]
                            gath = [
                                (ga, tabs[cs][:, :], isrc),
                                (gb, tabs[cd][:, :], idst),
                            ]
                            esz, estep = 2 * D, 2 * D
                        else:
                            ga = gap.tile([P, MAXG // P, D], mybir.dt.float32, tag="ga")
                            gb = gap.tile([P, MAXG // P, D], mybir.dt.float32, tag="gb")
                            ga_h = ga[:, :g, :]
                            gb_h = gb[:, :g, :]
                            gath = [
                                (ga, tabs[cs][:, 0:D], isrc),
                                (gb, tabs[cd][:, D:2 * D], idst),
                            ]
                            esz, estep = D, 2 * D
                        if mode not in ("B_dve",):
                            ns = SPLIT_GATHER if (g * P) % (SPLIT_GATHER * P) == 0 else 1
                            npart = n_idx // ns
                            for gt, src_ap, it in gath:
                                for si in range(ns):
                                    nc.gpsimd.dma_gather(
                                        gt[:, si * (g // ns):(si + 1) * (g // ns), :],
                                        src_ap,
                                        it[:, si * npart // 16:(si + 1) * npart // 16],
                                        npart, npart, esz, elem_step=estep,
                                        single_packet=SINGLE_PACKET, queue_num=gq[0],
                                    )
                                    gq[0] = (gq[0] + 1) % queues
                        # h = relu(ga + gb); logit = sum(h*w2) + b2
                        if mode in ("B_gather", "B_gp"):
                            idx_off += n_idx // 16
                            out_off += g
                            done += g
                            continue
                        lg = lgp.tile([P, MAXG // P], mybir.dt.float32, tag="lg")
                        if BF16:
                            # h in a separate CONTIGUOUS tile (unit-stride DVE,
                            # ga/gb release right after the add), then ONE
                            # fused DVE op does relu AND the +-1 sign multiply
                            # (|w2| pre-folded into the table), then a single
                            # full-width reduce.
                            h = hp.tile([P, MAXG // P, D], dt, tag="h")
                            ga_h = h[:, :g, :]
                            nc.vector.tensor_add(ga_h, ga[:, :g, 0:D], gb_h)
                            nc.vector.scalar_tensor_tensor(
                                ga_h, ga_h, 0.0,
                                w2rep_sb[:, :g * D].rearrange("p (s d) -> p s d", d=D),
                                op0=mybir.AluOpType.max,
                                op1=mybir.AluOpType.mult,
                            )
                            nc.vector.tensor_reduce(
                                lg[:, :g], ga_h,
                                axis=mybir.AxisListType.X,
                                op=mybir.AluOpType.add,
                            )
                        else:
                            nc.vector.tensor_add(ga_h, ga_h, gb_h)
                            nc.scalar.activation(
                                ga_h, ga_h, mybir.ActivationFunctionType.Relu,
                            )
                            nc.vector.tensor_mul(
                                ga_h, ga_h,
                                w2rep_sb[:, :g * D].rearrange("p (s d) -> p s d", d=D),
                            )
                            nc.vector.tensor_reduce(
                                lg[:, :g], ga_h,
                                axis=mybir.AxisListType.X, op=mybir.AluOpType.add,
                            )
                        nc.vector.tensor_scalar_add(
                            lg[:, :g], lg[:, :g], b2b_sb[:, :1]
                        )
                        nc.sync.dma_start(
                            out_t[:, out_off: out_off + g], lg[:, :g]
                        )
                        idx_off += n_idx // 16
                        out_off += g
                        done += g

                # schedule: chunks in order; after chunk c, all buckets whose
                # max(cs, cd) == c
                def emit_all(tabs):
                    nonlocal idx_off, out_off
                    idx_off = 0
                    out_off = 0
                    for c in range(N_CHUNKS):
                        precompute_chunk(c, tabs)
                        emitted_chunks.add(c)
                        for b in range(N_CHUNKS * N_CHUNKS):
                            cs, cd = b // N_CHUNKS, b % N_CHUNKS
                            if max(cs, cd) == c and s_b[b] > 0:
                                emit_bucket(b, tabs)

                def emit_buckets_only(tabs):
                    nonlocal idx_off, out_off
                    idx_off = 0
                    out_off = 0
                    for c in range(N_CHUNKS):
                        for b in range(N_CHUNKS * N_CHUNKS):
                            cs, cd = b // N_CHUNKS, b % N_CHUNKS
                            if max(cs, cd) == c and s_b[b] > 0:
                                emit_bucket(b, tabs)

                if loop_n is None:
                    emit_all(ab_sets[0])
                elif mode in ("full", "B_gp"):
                    with tc.For_i(0, loop_n, 1):
                        for abs_u in ab_sets:
                            emit_all(abs_u)
                else:
                    for c in range(N_CHUNKS):
                        precompute_chunk(c, ab_sets[0])
                    with tc.For_i(0, loop_n, 1):
                        for abs_u in ab_sets:
                            emit_buckets_only(abs_u)

    nc.compile()
    names = dict(
        embT=embT_t.name, w1cat=w1cat_t.name, w2rep=w2rep_t.name,
        b2b=b2b_t.name, isrc=isrc_t.name, idst=idst_t.name, out=out_t.name,
    )
    return nc, names, tot_slots


def prepare(nodes_emb, src, dst, W1, b1, W2, b2):
    """Host prep: bucket sort, index packing, input arrays. Returns a dict."""
    nodes_emb = np.ascontiguousarray(np.asarray(nodes_emb, dtype=np.float32))
    src = np.asarray(src).astype(np.int64)
    dst = np.asarray(dst).astype(np.int64)
    W1 = np.asarray(W1, dtype=np.float32)
    b1 = np.asarray(b1, dtype=np.float32).reshape(-1)
    W2 = np.asarray(W2, dtype=np.float32)
    b2 = np.asarray(b2, dtype=np.float32).reshape(-1)
    E = src.shape[0]
    N_CHUNKS, R_PAD = _derived()

    # ---- host prep -------------------------------------------------------
    # embT65 [65, R_PAD]: emb^T padded with zeros, plus a ones row (bias)
    embT = np.zeros((D + 1, R_PAD), dtype=np.float32)
    embT[:D, :N_NODES] = nodes_emb.T
    embT[D, :] = 1.0
    # w1cat65 [65, 128]: cols 0:64 -> A-half (W1[:64] with b1), 64:128 -> B-half
    w1cat = np.zeros((D + 1, 2 * D), dtype=np.float32)
    w1cat[:D, :D] = W1[:D]
    w1cat[:D, D:] = W1[D:]
    w1cat[D, :D] = b1            # bias folded into A-half
    w2rep = np.tile(W2.reshape(1, D), (P, 1)).astype(np.float32)
    b2b = np.full((P, 1), b2[0], dtype=np.float32)
    npos = D
    if BF16:
        import ml_dtypes
        # fold |w2| into the table columns; permute features so +sign w2
        # features come first (cols [0:npos]) and -sign after.
        w2v = W2.reshape(D)
        fperm = np.argsort(w2v < 0, kind="stable")     # positives first
        npos = int((w2v >= 0).sum())
        scale = np.abs(w2v[fperm])                     # >= 0
        w1cat = w1cat[:, np.concatenate([fperm, D + fperm])] * np.concatenate([scale, scale])
        # device matmul k covers table rows (k//M)*(M*128) + M*p + (k%M);
        # permute emb columns so table row r holds node r's data.
        M = PAIR_M
        j = np.arange(R_PAD)
        k, p_ = j // P, j % P
        perm = (k // M) * (M * P) + M * p_ + (k % M)
        embT = embT[:, perm].astype(ml_dtypes.bfloat16)
        w1cat = w1cat.astype(ml_dtypes.bfloat16)
        sgn = np.where(w2v[fperm] >= 0, 1.0, -1.0).astype(np.float32)
        w2rep = np.tile(sgn.reshape(1, D), (P, 1)).astype(ml_dtypes.bfloat16)

    # ---- bucket sort + deal over cores ----------------------------------
    cs = src // CHUNK
    cd = dst // CHUNK
    bucket = (cs * N_CHUNKS + cd).astype(np.int64)
    if SORT_SRC:
        # ascending src rows inside each bucket: gather descriptors hit
        # increasing HBM addresses (row-buffer friendly)
        order = np.lexsort((src, bucket))
    else:
        order = np.argsort(bucket, kind="stable")      # edge ids, bucket-major
    bcounts = np.bincount(bucket, minlength=N_CHUNKS * N_CHUNKS)
    # per-core-per-bucket count (round-robin deal), padded to 128
    m_bc = -(-bcounts // N_CORES)                      # ceil
    s_b = [int(_round_up(m, P) // P) if m > 0 else 0 for m in m_bc]
    tot_slots = int(sum(s_b))
    tot_idx = tot_slots * P

    src_s = (src[order] % CHUNK).astype(np.int16)
    dst_s = (dst[order] % CHUNK).astype(np.int16)

    # host index bookkeeping: for each sorted position, compute its
    # (core, flat device stream index)
    core_of = np.empty(E, dtype=np.int64)
    stream_of = np.empty(E, dtype=np.int64)
    bstart = np.concatenate([[0], np.cumsum(bcounts)])
    # device consumes buckets in emit order: bucket (cs, cd) is emitted after
    # AB chunks cs and cd, i.e. grouped by max(cs, cd)
    emit_order = [
        b for c in range(N_CHUNKS)
        for b in range(N_CHUNKS * N_CHUNKS)
        if max(b // N_CHUNKS, b % N_CHUNKS) == c
    ]
    slot_off = np.zeros(N_CHUNKS * N_CHUNKS, dtype=np.int64)
    acc = 0
    for b in emit_order:
        slot_off[b] = acc
        acc += s_b[b]
    for b in range(N_CHUNKS * N_CHUNKS):
        nb = bcounts[b]
        if nb == 0:
            continue
        pos = np.arange(nb)
        core_of[bstart[b]: bstart[b + 1]] = pos % N_CORES
        stream_of[bstart[b]: bstart[b + 1]] = slot_off[b] * P + pos // N_CORES

    isrc_all = np.zeros((N_CORES, tot_idx), dtype=np.int16)
    idst_all = np.zeros((N_CORES, tot_idx), dtype=np.int16)
    for c in range(N_CORES):
        m = core_of == c
        isrc_all[c, stream_of[m]] = src_s[m]
        idst_all[c, stream_of[m]] = dst_s[m]

    def wrap16(a):
        # stream index i -> [i % 16, i // 16], replicated to 128 partitions
        w = a.reshape(-1, 16).T                        # [16, tot/16]
        return np.tile(w, (8, 1)).copy()

    return dict(
        E=E, s_b=s_b, npos=npos, core_of=core_of, stream_of=stream_of,
        order=order, embT=embT, w1cat=w1cat, w2rep=w2rep, b2b=b2b,
        isrc=[wrap16(isrc_all[c]) for c in range(N_CORES)],
        idst=[wrap16(idst_all[c]) for c in range(N_CORES)],
    )


def make_in_maps(prep, names):
    return [
        {
            names["embT"]: prep["embT"],
            names["w1cat"]: prep["w1cat"],
            names["w2rep"]: prep["w2rep"],
            names["b2b"]: prep["b2b"],
            names["isrc"]: prep["isrc"][c],
            names["idst"]: prep["idst"][c],
        }
        for c in range(N_CORES)
    ]


def run_prep(prep, loop_n=None, unroll2=False):
    """Build the program (optionally with a repeat loop) and run it once."""
    from concourse.bass_utils import run_bass_kernel_spmd

    nc, names, tot_slots = _build_program(
        prep["s_b"], npos=prep.get("npos", D), loop_n=loop_n, queues=QUEUES,
        unroll2=unroll2,
    )
    in_maps = make_in_maps(prep, names)
    res = run_bass_kernel_spmd(
        nc, in_maps, core_ids=list(range(N_CORES)), trace=False,
    )
    global _last_results, _last_ctx, _last_names, _last_s_b
    _last_results = res
    _last_ctx = (nc, in_maps)
    _last_names = names
    _last_s_b = prep["s_b"]
    return res, names


def unscramble(prep, res, names):
    # device out [128, tot_slots]: stream index i -> out[i % 128, i // 128]
    E = prep["E"]
    core_of, stream_of, order = prep["core_of"], prep["stream_of"], prep["order"]
    logits_sorted = np.empty(E, dtype=np.float32)
    for c in range(N_CORES):
        o = res.results[c][names["out"]]               # [128, tot_slots]
        m = core_of == c
        si = stream_of[m]
        logits_sorted[np.flatnonzero(m)] = o[si % P, si // P]
    out = np.empty(E, dtype=np.float32)
    out[order] = logits_sorted
    return out.reshape(E, 1)


def kernel(nodes_emb, src, dst, W1, b1, W2, b2):
    prep = prepare(nodes_emb, src, dst, W1, b1, W2, b2)
    res, names = run_prep(prep, loop_n=None)
    return unscramble(prep, res, names)


def measure_hw(prep, r1=8, r2=64, n_iters=10, unroll2=False):
    """Differential HW timing: the program body repeated r inside one NEFF
    dispatch; per-execution time = (wall(r2) - wall(r1)) / (execs2 - execs1).

    The axon/PJRT dispatch overhead (~80 ms, validated with a trivial
    kernel) cancels in the difference.  With unroll2, each loop iteration
    holds TWO complete kernel executions on alternating DRAM table sets
    (removes the artificial write-after-read serialization at the loop
    back-edge that a single-shot run does not have).  Also verifies the
    loop programs produce the same outputs as the single-shot program.
    """
    mult = 2 if unroll2 else 1
    walls = {}
    outs = {}
    for r in (r1, r2):
        res, names = run_prep(prep, loop_n=r, unroll2=unroll2)
        outs[r] = unscramble(prep, res, names)
        ts = bench(n_iters=n_iters)
        walls[r] = min(ts)
        print(f"  loop_n={r} (x{mult}): wall min {walls[r]*1e3:.2f} ms "
              f"(iters: {[f'{t*1e3:.1f}' for t in ts]})")
    per_iter = (walls[r2] - walls[r1]) / ((r2 - r1) * mult)
    return per_iter, outs[r1], outs[r2]


def bench(n_iters=16, n_warmup=3):
    """Re-execute the last-compiled SPMD program on device-resident inputs.

    Returns list of per-iteration wall seconds (device exec + dispatch).
    """
    import time
    import jax
    import numpy as np
    from jax.sharding import Mesh, PartitionSpec
    from jax.experimental.shard_map import shard_map
    import concourse.mybir as mybir
    from concourse import bass2jax

    nc, in_maps = _last_ctx
    n_cores = len(in_maps)
    partition_name = nc.partition_id_tensor.name if nc.partition_id_tensor else None

    in_names, out_names, out_avals, zero_outs = [], [], [], []
    for alloc in nc.m.functions[0].allocations:
        if not isinstance(alloc, mybir.MemoryLocationSet):
            continue
        name = alloc.memorylocations[0].name
        if alloc.kind == "ExternalInput":
            if name != partition_name:
                in_names.append(name)
        elif alloc.kind == "ExternalOutput":
            shape = tuple(alloc.tensor_shape)
            dtype = mybir.dt.np(alloc.dtype)
            out_names.append(name)
            out_avals.append(jax.core.ShapedArray(shape, dtype))
            zero_outs.append(np.zeros(shape, dtype))
    n_params = len(in_names)
    n_outs = len(out_avals)
    in_names_all = in_names + out_names
    if partition_name is not None:
        in_names_all = in_names_all + [partition_name]

    def _body(*args):
        operands = list(args)
        if partition_name is not None:
            operands.append(bass2jax.partition_id_tensor())
        outs = bass2jax._bass_exec_p.bind(
            *operands,
            out_avals=tuple(out_avals),
            in_names=tuple(in_names_all),
            out_names=tuple(out_names),
            lowering_input_output_aliases=(),
            sim_require_finite=True,
            sim_require_nnan=True,
            nc=nc,
        )
        return tuple(outs)

    devices = jax.devices()[:n_cores]
    mesh = Mesh(np.asarray(devices), ("core",))
    in_specs = (PartitionSpec("core"),) * (n_params + n_outs)
    out_specs = (PartitionSpec("core"),) * n_outs
    donate = tuple(range(n_params, n_params + n_outs))
    sharded = jax.jit(
        shard_map(_body, mesh=mesh, in_specs=in_specs, out_specs=out_specs,
                  check_rep=False),
        donate_argnums=donate, keep_unused=True,
    )
    sharding = jax.sharding.NamedSharding(mesh, PartitionSpec("core"))
    concat_in = [
        jax.device_put(
            np.concatenate([np.asarray(in_maps[c][name]) for c in range(n_cores)], axis=0),
            sharding,
        )
        for name in in_names
    ]
    jax.block_until_ready(concat_in)
    n_total = n_warmup + n_iters
    zero_sets = [
        [
            jax.device_put(
                np.zeros((n_cores * z.shape[0], *z.shape[1:]), z.dtype), sharding
            )
            for z in zero_outs
        ]
        for _ in range(n_total)
    ]
    jax.block_until_ready(zero_sets)

    times = []
    for i in range(n_total):
        t0 = time.perf_counter()
        out = sharded(*concat_in, *zero_sets[i])
        jax.block_until_ready(out)
        times.append(time.perf_counter() - t0)
        del out
    return times[n_warmup:]



# revision 29
# speedup vs baseline: 1.1666x; 1.1077x over previous
"""GNN edge-scorer kernel for Trainium2 (8 NeuronCores, SPMD).

reference:
    edge_emb = concat(emb[src], emb[dst])          # [E, 128]
    h = relu(edge_emb @ W1 + b1)                   # [E, 64]
    logits = h @ W2 + b2                           # [E, 1]

Device algorithm (memory-bound gather regime; BF16=True path):
  Phase A (per core, replicated): AB[n] = [emb[n]@W1[:64]*|w2| + b1*|w2| |
    emb[n]@W1[64:]*|w2|] via PE matmul with K=65 (ones-row folds b1), cast
    to bf16.  AB is [150016, 128] bf16 in DRAM, row = 256 B.  |w2| is folded
    into the table (relu(k x) = k relu(x), k>=0).  Table writes put PAIR_M=8
    consecutive rows on one partition -> 2 KB write descriptors (table padded
    to a 1024-multiple row count).
  Phase B: per edge, dma_gather full 256 B rows of AB[src] and AB[dst]
    (int16 idx, elem_step=128), each gather split across 2 SWDGE queues
    (queue-level parallelism raises SDMA gather throughput ~35%); DVE:
    h = add, then ONE fused scalar_tensor_tensor op = relu x (+-1 sign of
    w2), then a single full-width reduce: logit = sum + b2.  The gather is
    descriptor-rate-bound (measured; address locality does not matter), so
    everything else overlaps behind it.

  dma_gather int16 indices only span 32768 rows -> table split into 5 chunks;
  host sorts edges into 25 (src_chunk, dst_chunk) buckets and deals them
  round-robin over the 8 cores so every core has identical bucket sizes
  (single SPMD program).  Output unscrambled on host.
"""

import numpy as np

N_NODES = 150000
TRACE = False            # set by test harness to capture HW profile
_last_results = None     # BassKernelResults of the most recent run
_last_ctx = None         # (nc, in_maps) of the most recent run
_last_names = None
_last_s_b = None
D = 64
P = 128
N_CORES = 8
CHUNK = 32768                      # dma_gather int16 index range
NT = 1024                          # precompute nodes per tile
MAXG = 8192                        # max indices per dma_gather instruction
QUEUES = 4                         # SWDGE queues for gather desc-gen
SINGLE_PACKET = False              # dma_gather single_packet flag
SORT_SRC = False                   # secondary-sort edges by src within bucket
PAIR_M = 8                         # nodes per partition per AB-write descriptor (bf16)
SPLIT_GATHER = 2                   # split each dma_gather into this many queue-parallel parts


def _derived():
    n_chunks = (N_NODES + CHUNK - 1) // CHUNK
    # pad so every chunk (incl. the last) splits into full NT=1024 tiles:
    # PAIR_M=8 write grouping needs n % (PAIR_M*128) == 0 per tile
    r_pad = CHUNK * (n_chunks - 1) + NT * (
        (N_NODES - CHUNK * (n_chunks - 1) + NT - 1) // NT
    )
    return n_chunks, r_pad


def _round_up(x, m):
    return (x + m - 1) // m * m


BF16 = True


def _build_program(s_b, npos=D, loop_n=None, mode="full", queues=1, unroll2=False, b2_zero=False):
    """Build the SPMD Bass program.

    s_b: list of 25 per-bucket slot counts (slots of 128 edges, same on
    every core).  Returns (nc, names dict).

    BF16=True: AB table stored bf16 (row = 128 bf16 = 256B); gathers pull
    the full row for both endpoints and the compute uses the A half of the
    src row and the B half of the dst row.  Precompute writes pair nodes
    (2p, 2p+1) per partition so each write descriptor is 512B.

    |w2| is folded into the table columns on the host (relu(k*x) = k*relu(x)
    for k>=0) and features are permuted so cols [0:npos] carry positive w2
    sign and [npos:D] negative: logit = sum(relu[0:npos]) - sum(relu[npos:D])
    + b2, eliminating the per-edge w2 multiply.
    """
    import concourse.bacc as bacc
    import concourse.mybir as mybir
    import concourse.tile as tile

    N_CHUNKS, R_PAD = _derived()
    tot_slots = int(sum(s_b))
    tot_idx = tot_slots * P
    dt = mybir.dt.bfloat16 if BF16 else mybir.dt.float32

    nc = bacc.Bacc(None, target_bir_lowering=False, debug=False, num_swdge_queues=queues)
    with tile.TileContext(nc) as tc:
        with tc.tile_pool(name="dram", bufs=1, space="DRAM") as dram:
            embT_t = dram.tile([D + 1, R_PAD], dt, kind="ExternalInput")
            w1cat_t = dram.tile([D + 1, 2 * D], dt, kind="ExternalInput")
            w2rep_t = dram.tile([P, D], dt, kind="ExternalInput")
            b2b_t = dram.tile([P, 1], mybir.dt.float32, kind="ExternalInput")
            isrc_t = dram.tile([P, tot_idx // 16], mybir.dt.int16, kind="ExternalInput")
            idst_t = dram.tile([P, tot_idx // 16], mybir.dt.int16, kind="ExternalInput")
            out_t = dram.tile([P, tot_slots], mybir.dt.float32, kind="ExternalOutput")

            # AB table, one DRAM tile per 32768-row chunk so Tile can
            # pipeline gathers of chunk c behind the writes of chunk c.
            ab_rows = [CHUNK] * (N_CHUNKS - 1) + [R_PAD - CHUNK * (N_CHUNKS - 1)]
            ab_sets = [[
                dram.tile([r, 2 * D], dt, name=f"ab{c}_{u}")
                for c, r in enumerate(ab_rows)
            ] for u in range(2 if unroll2 else 1)]

            with (
                tc.tile_pool(name="consts", bufs=1) as consts,
                tc.tile_pool(name="pre_et", bufs=3) as pre_et,
                tc.tile_pool(name="pre_ab", bufs=3) as pre_ab,
                tc.tile_pool(name="ps_ab", bufs=2, space="PSUM") as ps_ab,
                tc.tile_pool(name="idx", bufs=4) as idxp,
                tc.tile_pool(name="ga", bufs=4 if BF16 else 3) as gap,
                tc.tile_pool(name="h", bufs=4) as hp,
                tc.tile_pool(name="lg", bufs=4) as lgp,
            ):
                w1cat_sb = consts.tile([D + 1, 2 * D], dt)
                nc.sync.dma_start(w1cat_sb[:], w1cat_t[:])
                b2b_sb = consts.tile([P, 1], mybir.dt.float32)
                nc.sync.dma_start(b2b_sb[:], b2b_t[:])
                # BF16: w2rep_t carries the +-1 sign vector (|w2| is folded
                # into the table); f32: it carries w2 itself.  Either way,
                # replicate across MAXG//128 slots -> [P, (MAXG//P)*D].
                w2in_sb = consts.tile([P, D], dt, tag="w2in")
                nc.sync.dma_start(w2in_sb[:], w2rep_t[:])
                w2rep_sb = consts.tile([P, (MAXG // P) * D], dt, tag="w2rep")
                for j in range(MAXG // P):
                    nc.vector.tensor_copy(w2rep_sb[:, j * D:(j + 1) * D], w2in_sb[:])

                # ---- interleaved emission: precompute chunks + buckets ----
                def precompute_chunk(c, tabs):
                    base = c * CHUNK
                    rows = ab_rows[c]
                    for nt0 in range(0, rows, NT):
                        n = min(NT, rows - nt0)
                        et = pre_et.tile([D + 1, NT], dt, tag="et")
                        nc.sync.dma_start(
                            et[:, :n], embT_t[:, base + nt0: base + nt0 + n]
                        )
                        ab_sb = pre_ab.tile([P, NT // P * 2 * D], dt, tag="absb")
                        assert n % (4 * P) == 0
                        for k4 in range(n // (4 * P)):
                            # 4 matmuls fill one PSUM bank -> one wide copy
                            ab_ps = ps_ab.tile([P, 512], mybir.dt.float32, space="PSUM")
                            for m in range(4):
                                k = 4 * k4 + m
                                nc.tensor.matmul(
                                    ab_ps[:, m * 2 * D:(m + 1) * 2 * D],
                                    lhsT=et[:, k * P:(k + 1) * P],
                                    rhs=w1cat_sb[:],
                                    start=True, stop=True,
                                )
                            nc.scalar.activation(
                                ab_sb[:, k4 * 512:(k4 + 1) * 512],
                                ab_ps[:],
                                mybir.ActivationFunctionType.Copy,
                            )
                        if BF16:
                            # host permuted emb columns so matmul k covers
                            # device nodes {nt0 + (k//M)*(M*128) + M*p + k%M}:
                            # rows M*p..M*p+M-1 of each group sit on partition
                            # p -> M*256B contiguous write descriptors.
                            M = PAIR_M
                            assert n % (M * P) == 0
                            dst = tabs[c][nt0:nt0 + n, :].rearrange(
                                "(g p m) f -> p g m f", p=P, m=M
                            )
                            src_ap = ab_sb[:, :n // P * 2 * D].rearrange(
                                "p (g m f) -> p g m f", m=M, f=2 * D
                            )
                        else:
                            # AB rows nt0+k*128+p , feat f  <- ab_sb[p, k*128+f]
                            dst = tabs[c][nt0:nt0 + n, :].rearrange(
                                "(k p) f -> p k f", p=P
                            )
                            src_ap = ab_sb[:, :n // P * 2 * D].rearrange(
                                "p (k f) -> p k f", f=2 * D
                            )
                        nc.sync.dma_start(dst, src_ap)

                # bucket schedule: emit bucket (cs,cd) after chunks cs, cd
                emitted_chunks = set()
                idx_off = 0   # in int16 columns (tot_idx//16 total)
                out_off = 0   # in slots

                gq = [0]

                def emit_bucket(b, tabs):
                    nonlocal idx_off, out_off
                    cs, cd = b // N_CHUNKS, b % N_CHUNKS
                    slots = s_b[b]
                    done = 0
                    while done < slots:
                        g = min(MAXG // P, slots - done)      # slots this gather
                        n_idx = g * P
                        isrc = idxp.tile([P, MAXG // 16], mybir.dt.int16, tag="isrc")
                        idst = idxp.tile([P, MAXG // 16], mybir.dt.int16, tag="idst")
                        nc.sync.dma_start(
                            isrc[:, :n_idx // 16],
                            isrc_t[:, idx_off: idx_off + n_idx // 16],
                        )
                        nc.sync.dma_start(
                            idst[:, :n_idx // 16],
                            idst_t[:, idx_off: idx_off + n_idx // 16],
                        )
                        if BF16:
                            # full 256B rows; use A half of src, B half of dst
                            ga = gap.tile([P, MAXG // P, 2 * D], dt, tag="ga")
                            gb = gap.tile([P, MAXG // P, 2 * D], dt, tag="gb")
                            ga_h = ga[:, :g, 0:D]
                            gb_h = gb[:, :g, D:2 * D]
                            gath = [
                                (ga, tabs[cs][:, :], isrc),
                                (gb, tabs[cd][:, :], idst),
                            ]
                            esz, estep = 2 * D, 2 * D
                        else:
                            ga = gap.tile([P, MAXG // P, D], mybir.dt.float32, tag="ga")
                            gb = gap.tile([P, MAXG // P, D], mybir.dt.float32, tag="gb")
                            ga_h = ga[:, :g, :]
                            gb_h = gb[:, :g, :]
                            gath = [
                                (ga, tabs[cs][:, 0:D], isrc),
                                (gb, tabs[cd][:, D:2 * D], idst),
                            ]
                            esz, estep = D, 2 * D
                        if mode not in ("B_dve",):
                            ns = SPLIT_GATHER if (g * P) % (SPLIT_GATHER * P) == 0 else 1
                            npart = n_idx // ns
                            for gt, src_ap, it in gath:
                                for si in range(ns):
                                    nc.gpsimd.dma_gather(
                                        gt[:, si * (g // ns):(si + 1) * (g // ns), :],
                                        src_ap,
                                        it[:, si * npart // 16:(si + 1) * npart // 16],
                                        npart, npart, esz, elem_step=estep,
                                        single_packet=SINGLE_PACKET, queue_num=gq[0],
                                    )
                                    gq[0] = (gq[0] + 1) % queues
                        # h = relu(ga + gb); logit = sum(h*w2) + b2
                        if mode in ("B_gather", "B_gp"):
                            idx_off += n_idx // 16
                            out_off += g
                            done += g
                            continue
                        lg = lgp.tile([P, MAXG // P], mybir.dt.float32, tag="lg")
                        if BF16:
                            # h in a separate CONTIGUOUS tile (unit-stride DVE,
                            # ga/gb release right after the add), then ONE
                            # fused DVE op does relu AND the +-1 sign multiply
                            # (|w2| pre-folded into the table), then a single
                            # full-width reduce.
                            h = hp.tile([P, MAXG // P, D], dt, tag="h")
                            ga_h = h[:, :g, :]
                            nc.vector.tensor_add(ga_h, ga[:, :g, 0:D], gb_h)
                            nc.vector.scalar_tensor_tensor(
                                ga_h, ga_h, 0.0,
                                w2rep_sb[:, :g * D].rearrange("p (s d) -> p s d", d=D),
                                op0=mybir.AluOpType.max,
                                op1=mybir.AluOpType.mult,
                            )
                            nc.vector.tensor_reduce(
                                lg[:, :g], ga_h,
                                axis=mybir.AxisListType.X,
                                op=mybir.AluOpType.add,
                            )
                        else:
                            nc.vector.tensor_add(ga_h, ga_h, gb_h)
                            nc.scalar.activation(
                                ga_h, ga_h, mybir.ActivationFunctionType.Relu,
                            )
                            nc.vector.tensor_mul(
                                ga_h, ga_h,
                                w2rep_sb[:, :g * D].rearrange("p (s d) -> p s d", d=D),
                            )
                            nc.vector.tensor_reduce(
                                lg[:, :g], ga_h,
                                axis=mybir.AxisListType.X, op=mybir.AluOpType.add,
                            )
                        if not b2_zero:
                            nc.vector.tensor_scalar_add(
                                lg[:, :g], lg[:, :g], b2b_sb[:, :1]
                            )
                        nc.sync.dma_start(
                            out_t[:, out_off: out_off + g], lg[:, :g]
                        )
                        idx_off += n_idx // 16
                        out_off += g
                        done += g

                # schedule: chunks in order; after chunk c, all buckets whose
                # max(cs, cd) == c
                def emit_all(tabs):
                    nonlocal idx_off, out_off
                    idx_off = 0
                    out_off = 0
                    for c in range(N_CHUNKS):
                        precompute_chunk(c, tabs)
                        emitted_chunks.add(c)
                        for b in range(N_CHUNKS * N_CHUNKS):
                            cs, cd = b // N_CHUNKS, b % N_CHUNKS
                            if max(cs, cd) == c and s_b[b] > 0:
                                emit_bucket(b, tabs)

                def emit_buckets_only(tabs):
                    nonlocal idx_off, out_off
                    idx_off = 0
                    out_off = 0
                    for c in range(N_CHUNKS):
                        for b in range(N_CHUNKS * N_CHUNKS):
                            cs, cd = b // N_CHUNKS, b % N_CHUNKS
                            if max(cs, cd) == c and s_b[b] > 0:
                                emit_bucket(b, tabs)

                if loop_n is None:
                    emit_all(ab_sets[0])
                elif mode in ("full", "B_gp"):
                    with tc.For_i(0, loop_n, 1):
                        for abs_u in ab_sets:
                            emit_all(abs_u)
                else:
                    for c in range(N_CHUNKS):
                        precompute_chunk(c, ab_sets[0])
                    with tc.For_i(0, loop_n, 1):
                        for abs_u in ab_sets:
                            emit_buckets_only(abs_u)

    nc.compile()
    names = dict(
        embT=embT_t.name, w1cat=w1cat_t.name, w2rep=w2rep_t.name,
        b2b=b2b_t.name, isrc=isrc_t.name, idst=idst_t.name, out=out_t.name,
    )
    return nc, names, tot_slots


def prepare(nodes_emb, src, dst, W1, b1, W2, b2):
    """Host prep: bucket sort, index packing, input arrays. Returns a dict."""
    nodes_emb = np.ascontiguousarray(np.asarray(nodes_emb, dtype=np.float32))
    src = np.asarray(src).astype(np.int64)
    dst = np.asarray(dst).astype(np.int64)
    W1 = np.asarray(W1, dtype=np.float32)
    b1 = np.asarray(b1, dtype=np.float32).reshape(-1)
    W2 = np.asarray(W2, dtype=np.float32)
    b2 = np.asarray(b2, dtype=np.float32).reshape(-1)
    E = src.shape[0]
    N_CHUNKS, R_PAD = _derived()

    # ---- host prep -------------------------------------------------------
    # embT65 [65, R_PAD]: emb^T padded with zeros, plus a ones row (bias)
    embT = np.zeros((D + 1, R_PAD), dtype=np.float32)
    embT[:D, :N_NODES] = nodes_emb.T
    embT[D, :] = 1.0
    # w1cat65 [65, 128]: cols 0:64 -> A-half (W1[:64] with b1), 64:128 -> B-half
    w1cat = np.zeros((D + 1, 2 * D), dtype=np.float32)
    w1cat[:D, :D] = W1[:D]
    w1cat[:D, D:] = W1[D:]
    w1cat[D, :D] = b1            # bias folded into A-half
    w2rep = np.tile(W2.reshape(1, D), (P, 1)).astype(np.float32)
    b2b = np.full((P, 1), b2[0], dtype=np.float32)
    npos = D
    if BF16:
        import ml_dtypes
        # fold |w2| into the table columns; permute features so +sign w2
        # features come first (cols [0:npos]) and -sign after.
        w2v = W2.reshape(D)
        fperm = np.argsort(w2v < 0, kind="stable")     # positives first
        npos = int((w2v >= 0).sum())
        scale = np.abs(w2v[fperm])                     # >= 0
        w1cat = w1cat[:, np.concatenate([fperm, D + fperm])] * np.concatenate([scale, scale])
        # device matmul k covers table rows (k//M)*(M*128) + M*p + (k%M);
        # permute emb columns so table row r holds node r's data.
        M = PAIR_M
        j = np.arange(R_PAD)
        k, p_ = j // P, j % P
        perm = (k // M) * (M * P) + M * p_ + (k % M)
        embT = embT[:, perm].astype(ml_dtypes.bfloat16)
        w1cat = w1cat.astype(ml_dtypes.bfloat16)
        sgn = np.where(w2v[fperm] >= 0, 1.0, -1.0).astype(np.float32)
        w2rep = np.tile(sgn.reshape(1, D), (P, 1)).astype(ml_dtypes.bfloat16)

    # ---- bucket sort + deal over cores ----------------------------------
    cs = src // CHUNK
    cd = dst // CHUNK
    bucket = (cs * N_CHUNKS + cd).astype(np.int64)
    if SORT_SRC:
        # ascending src rows inside each bucket: gather descriptors hit
        # increasing HBM addresses (row-buffer friendly)
        order = np.lexsort((src, bucket))
    else:
        order = np.argsort(bucket, kind="stable")      # edge ids, bucket-major
    bcounts = np.bincount(bucket, minlength=N_CHUNKS * N_CHUNKS)
    # per-core-per-bucket count (round-robin deal), padded to 128
    m_bc = -(-bcounts // N_CORES)                      # ceil
    s_b = [int(_round_up(m, P) // P) if m > 0 else 0 for m in m_bc]
    tot_slots = int(sum(s_b))
    tot_idx = tot_slots * P

    src_s = (src[order] % CHUNK).astype(np.int16)
    dst_s = (dst[order] % CHUNK).astype(np.int16)

    # host index bookkeeping: for each sorted position, compute its
    # (core, flat device stream index)
    core_of = np.empty(E, dtype=np.int64)
    stream_of = np.empty(E, dtype=np.int64)
    bstart = np.concatenate([[0], np.cumsum(bcounts)])
    # device consumes buckets in emit order: bucket (cs, cd) is emitted after
    # AB chunks cs and cd, i.e. grouped by max(cs, cd)
    emit_order = [
        b for c in range(N_CHUNKS)
        for b in range(N_CHUNKS * N_CHUNKS)
        if max(b // N_CHUNKS, b % N_CHUNKS) == c
    ]
    slot_off = np.zeros(N_CHUNKS * N_CHUNKS, dtype=np.int64)
    acc = 0
    for b in emit_order:
        slot_off[b] = acc
        acc += s_b[b]
    for b in range(N_CHUNKS * N_CHUNKS):
        nb = bcounts[b]
        if nb == 0:
            continue
        pos = np.arange(nb)
        core_of[bstart[b]: bstart[b + 1]] = pos % N_CORES
        stream_of[bstart[b]: bstart[b + 1]] = slot_off[b] * P + pos // N_CORES

    isrc_all = np.zeros((N_CORES, tot_idx), dtype=np.int16)
    idst_all = np.zeros((N_CORES, tot_idx), dtype=np.int16)
    for c in range(N_CORES):
        m = core_of == c
        isrc_all[c, stream_of[m]] = src_s[m]
        idst_all[c, stream_of[m]] = dst_s[m]

    def wrap16(a):
        # stream index i -> [i % 16, i // 16], replicated to 128 partitions
        w = a.reshape(-1, 16).T                        # [16, tot/16]
        return np.tile(w, (8, 1)).copy()

    return dict(
        E=E, s_b=s_b, npos=npos, b2_zero=bool(b2[0] == 0.0),
        core_of=core_of, stream_of=stream_of,
        order=order, embT=embT, w1cat=w1cat, w2rep=w2rep, b2b=b2b,
        isrc=[wrap16(isrc_all[c]) for c in range(N_CORES)],
        idst=[wrap16(idst_all[c]) for c in range(N_CORES)],
    )


def make_in_maps(prep, names):
    return [
        {
            names["embT"]: prep["embT"],
            names["w1cat"]: prep["w1cat"],
            names["w2rep"]: prep["w2rep"],
            names["b2b"]: prep["b2b"],
            names["isrc"]: prep["isrc"][c],
            names["idst"]: prep["idst"][c],
        }
        for c in range(N_CORES)
    ]


def run_prep(prep, loop_n=None, unroll2=False):
    """Build the program (optionally with a repeat loop) and run it once."""
    from concourse.bass_utils import run_bass_kernel_spmd

    nc, names, tot_slots = _build_program(
        prep["s_b"], npos=prep.get("npos", D), loop_n=loop_n, queues=QUEUES,
        unroll2=unroll2, b2_zero=prep.get("b2_zero", False),
    )
    in_maps = make_in_maps(prep, names)
    res = run_bass_kernel_spmd(
        nc, in_maps, core_ids=list(range(N_CORES)), trace=False,
    )
    global _last_results, _last_ctx, _last_names, _last_s_b
    _last_results = res
    _last_ctx = (nc, in_maps)
    _last_names = names
    _last_s_b = prep["s_b"]
    return res, names


def unscramble(prep, res, names):
    # device out [128, tot_slots]: stream index i -> out[i % 128, i // 128]
    E = prep["E"]
    core_of, stream_of, order = prep["core_of"], prep["stream_of"], prep["order"]
    logits_sorted = np.empty(E, dtype=np.float32)
    for c in range(N_CORES):
        o = res.results[c][names["out"]]               # [128, tot_slots]
        m = core_of == c
        si = stream_of[m]
        logits_sorted[np.flatnonzero(m)] = o[si % P, si // P]
    out = np.empty(E, dtype=np.float32)
    out[order] = logits_sorted
    return out.reshape(E, 1)


def kernel(nodes_emb, src, dst, W1, b1, W2, b2):
    prep = prepare(nodes_emb, src, dst, W1, b1, W2, b2)
    res, names = run_prep(prep, loop_n=None)
    return unscramble(prep, res, names)


def measure_hw(prep, r1=8, r2=64, n_iters=10, unroll2=False):
    """Differential HW timing: the program body repeated r inside one NEFF
    dispatch; per-execution time = (wall(r2) - wall(r1)) / (execs2 - execs1).

    The axon/PJRT dispatch overhead (~80 ms, validated with a trivial
    kernel) cancels in the difference.  With unroll2, each loop iteration
    holds TWO complete kernel executions on alternating DRAM table sets
    (removes the artificial write-after-read serialization at the loop
    back-edge that a single-shot run does not have).  Also verifies the
    loop programs produce the same outputs as the single-shot program.
    """
    mult = 2 if unroll2 else 1
    walls = {}
    outs = {}
    for r in (r1, r2):
        res, names = run_prep(prep, loop_n=r, unroll2=unroll2)
        outs[r] = unscramble(prep, res, names)
        ts = bench(n_iters=n_iters)
        walls[r] = min(ts)
        print(f"  loop_n={r} (x{mult}): wall min {walls[r]*1e3:.2f} ms "
              f"(iters: {[f'{t*1e3:.1f}' for t in ts]})")
    per_iter = (walls[r2] - walls[r1]) / ((r2 - r1) * mult)
    return per_iter, outs[r1], outs[r2]


def bench(n_iters=16, n_warmup=3):
    """Re-execute the last-compiled SPMD program on device-resident inputs.

    Returns list of per-iteration wall seconds (device exec + dispatch).
    """
    import time
    import jax
    import numpy as np
    from jax.sharding import Mesh, PartitionSpec
    from jax.experimental.shard_map import shard_map
    import concourse.mybir as mybir
    from concourse import bass2jax

    nc, in_maps = _last_ctx
    n_cores = len(in_maps)
    partition_name = nc.partition_id_tensor.name if nc.partition_id_tensor else None

    in_names, out_names, out_avals, zero_outs = [], [], [], []
    for alloc in nc.m.functions[0].allocations:
        if not isinstance(alloc, mybir.MemoryLocationSet):
            continue
        name = alloc.memorylocations[0].name
        if alloc.kind == "ExternalInput":
            if name != partition_name:
                in_names.append(name)
        elif alloc.kind == "ExternalOutput":
            shape = tuple(alloc.tensor_shape)
            dtype = mybir.dt.np(alloc.dtype)
            out_names.append(name)
            out_avals.append(jax.core.ShapedArray(shape, dtype))
            zero_outs.append(np.zeros(shape, dtype))
    n_params = len(in_names)
    n_outs = len(out_avals)
    in_names_all = in_names + out_names
    if partition_name is not None:
        in_names_all = in_names_all + [partition_name]

    def _body(*args):
        operands = list(args)
        if partition_name is not None:
            operands.append(bass2jax.partition_id_tensor())
        outs = bass2jax._bass_exec_p.bind(
            *operands,
            out_avals=tuple(out_avals),
            in_names=tuple(in_names_all),
            out_names=tuple(out_names),
            lowering_input_output_aliases=(),
            sim_require_finite=True,
            sim_require_nnan=True,
            nc=nc,
        )
        return tuple(outs)

    devices = jax.devices()[:n_cores]
    mesh = Mesh(np.asarray(devices), ("core",))
    in_specs = (PartitionSpec("core"),) * (n_params + n_outs)
    out_specs = (PartitionSpec("core"),) * n_outs
    donate = tuple(range(n_params, n_params + n_outs))
    sharded = jax.jit(
        shard_map(_body, mesh=mesh, in_specs=in_specs, out_specs=out_specs,
                  check_rep=False),
        donate_argnums=donate, keep_unused=True,
    )
    sharding = jax.sharding.NamedSharding(mesh, PartitionSpec("core"))
    concat_in = [
        jax.device_put(
            np.concatenate([np.asarray(in_maps[c][name]) for c in range(n_cores)], axis=0),
            sharding,
        )
        for name in in_names
    ]
    jax.block_until_ready(concat_in)
    n_total = n_warmup + n_iters
    zero_sets = [
        [
            jax.device_put(
                np.zeros((n_cores * z.shape[0], *z.shape[1:]), z.dtype), sharding
            )
            for z in zero_outs
        ]
        for _ in range(n_total)
    ]
    jax.block_until_ready(zero_sets)

    times = []
    for i in range(n_total):
        t0 = time.perf_counter()
        out = sharded(*concat_in, *zero_sets[i])
        jax.block_until_ready(out)
        times.append(time.perf_counter() - t0)
        del out
    return times[n_warmup:]



# revision 30
# speedup vs baseline: 1.2148x; 1.0413x over previous
"""GNN edge-scorer kernel for Trainium2 (8 NeuronCores, SPMD).

reference:
    edge_emb = concat(emb[src], emb[dst])          # [E, 128]
    h = relu(edge_emb @ W1 + b1)                   # [E, 64]
    logits = h @ W2 + b2                           # [E, 1]

Device algorithm (memory-bound gather regime; BF16=True path):
  Phase A (per core, replicated): AB[n] = [emb[n]@W1[:64]*|w2| + b1*|w2| |
    emb[n]@W1[64:]*|w2|] via PE matmul with K=65 (ones-row folds b1), cast
    to bf16.  AB is [150016, 128] bf16 in DRAM, row = 256 B.  |w2| is folded
    into the table (relu(k x) = k relu(x), k>=0).  Table writes put PAIR_M=8
    consecutive rows on one partition -> 2 KB write descriptors (table padded
    to a 1024-multiple row count).
  Phase B: per edge, dma_gather full 256 B rows of AB[src] and AB[dst]
    (int16 idx, elem_step=128), each gather split across 2 SWDGE queues
    (queue-level parallelism raises SDMA gather throughput ~35%); DVE:
    h = add, then ONE fused scalar_tensor_tensor op = relu x (+-1 sign of
    w2), then a single full-width reduce: logit = sum + b2.  The gather is
    descriptor-rate-bound (measured; address locality does not matter), so
    everything else overlaps behind it.

  dma_gather int16 indices only span 32768 rows -> table split into 5 chunks;
  host sorts edges into 25 (src_chunk, dst_chunk) buckets and deals them
  round-robin over the 8 cores so every core has identical bucket sizes
  (single SPMD program).  Output unscrambled on host.
"""

import numpy as np

N_NODES = 150000
TRACE = False            # set by test harness to capture HW profile
_last_results = None     # BassKernelResults of the most recent run
_last_ctx = None         # (nc, in_maps) of the most recent run
_last_names = None
_last_s_b = None
D = 64
P = 128
N_CORES = 8
CHUNK = 32768                      # dma_gather int16 index range
NT = 1024                          # precompute nodes per tile
MAXG = 8192                        # max indices per dma_gather instruction
QUEUES = 4                         # SWDGE queues for gather desc-gen
SINGLE_PACKET = False              # dma_gather single_packet flag
SORT_SRC = False                   # secondary-sort edges by src within bucket
PAIR_M = 8                         # nodes per partition per AB-write descriptor (bf16)
SPLIT_GATHER = 2                   # split each dma_gather into this many queue-parallel parts


def _derived():
    n_chunks = (N_NODES + CHUNK - 1) // CHUNK
    # pad so every chunk (incl. the last) splits into full NT=1024 tiles:
    # PAIR_M=8 write grouping needs n % (PAIR_M*128) == 0 per tile
    r_pad = CHUNK * (n_chunks - 1) + NT * (
        (N_NODES - CHUNK * (n_chunks - 1) + NT - 1) // NT
    )
    return n_chunks, r_pad


def _round_up(x, m):
    return (x + m - 1) // m * m


BF16 = True


def _build_program(s_b, npos=D, loop_n=None, mode="full", queues=1, unroll2=False, b2_zero=False):
    """Build the SPMD Bass program.

    s_b: list of 25 per-bucket slot counts (slots of 128 edges, same on
    every core).  Returns (nc, names dict).

    BF16=True: AB table stored bf16 (row = 128 bf16 = 256B); gathers pull
    the full row for both endpoints and the compute uses the A half of the
    src row and the B half of the dst row.  Precompute writes pair nodes
    (2p, 2p+1) per partition so each write descriptor is 512B.

    |w2| is folded into the table columns on the host (relu(k*x) = k*relu(x)
    for k>=0) and features are permuted so cols [0:npos] carry positive w2
    sign and [npos:D] negative: logit = sum(relu[0:npos]) - sum(relu[npos:D])
    + b2, eliminating the per-edge w2 multiply.
    """
    import concourse.bacc as bacc
    import concourse.mybir as mybir
    import concourse.tile as tile

    N_CHUNKS, R_PAD = _derived()
    tot_slots = int(sum(s_b))
    tot_idx = tot_slots * P
    dt = mybir.dt.bfloat16 if BF16 else mybir.dt.float32

    nc = bacc.Bacc(None, target_bir_lowering=False, debug=False, num_swdge_queues=queues)
    with tile.TileContext(nc) as tc:
        with tc.tile_pool(name="dram", bufs=1, space="DRAM") as dram:
            embT_t = dram.tile([D + 1, R_PAD], dt, kind="ExternalInput")
            w1cat_t = dram.tile([D + 1, 2 * D], dt, kind="ExternalInput")
            w2rep_t = dram.tile([P, D], dt, kind="ExternalInput")
            b2b_t = dram.tile([P, 1], mybir.dt.float32, kind="ExternalInput")
            isrc_t = dram.tile([P, tot_idx // 16], mybir.dt.int16, kind="ExternalInput")
            idst_t = dram.tile([P, tot_idx // 16], mybir.dt.int16, kind="ExternalInput")
            out_t = dram.tile([P, tot_slots], mybir.dt.float32, kind="ExternalOutput")

            # AB table, one DRAM tile per 32768-row chunk so Tile can
            # pipeline gathers of chunk c behind the writes of chunk c.
            ab_rows = [CHUNK] * (N_CHUNKS - 1) + [R_PAD - CHUNK * (N_CHUNKS - 1)]
            ab_sets = [[
                dram.tile([r, 2 * D], dt, name=f"ab{c}_{u}")
                for c, r in enumerate(ab_rows)
            ] for u in range(2 if unroll2 else 1)]

            with (
                tc.tile_pool(name="consts", bufs=1) as consts,
                tc.tile_pool(name="pre_et", bufs=3) as pre_et,
                tc.tile_pool(name="pre_ab", bufs=3) as pre_ab,
                tc.tile_pool(name="ps_ab", bufs=2, space="PSUM") as ps_ab,
                tc.tile_pool(name="idx", bufs=4) as idxp,
                tc.tile_pool(name="ga", bufs=4 if BF16 else 3) as gap,
                tc.tile_pool(name="h", bufs=4) as hp,
                tc.tile_pool(name="lg", bufs=4) as lgp,
            ):
                lgall = consts.tile([P, tot_slots], mybir.dt.float32, tag="lgall")
                w1cat_sb = consts.tile([D + 1, 2 * D], dt)
                nc.sync.dma_start(w1cat_sb[:], w1cat_t[:])
                b2b_sb = consts.tile([P, 1], mybir.dt.float32)
                nc.sync.dma_start(b2b_sb[:], b2b_t[:])
                # BF16: w2rep_t carries the +-1 sign vector (|w2| is folded
                # into the table); f32: it carries w2 itself.  Either way,
                # replicate across MAXG//128 slots -> [P, (MAXG//P)*D].
                w2in_sb = consts.tile([P, D], dt, tag="w2in")
                nc.sync.dma_start(w2in_sb[:], w2rep_t[:])
                w2rep_sb = consts.tile([P, (MAXG // P) * D], dt, tag="w2rep")
                for j in range(MAXG // P):
                    nc.vector.tensor_copy(w2rep_sb[:, j * D:(j + 1) * D], w2in_sb[:])

                # ---- interleaved emission: precompute chunks + buckets ----
                def precompute_chunk(c, tabs):
                    base = c * CHUNK
                    rows = ab_rows[c]
                    for nt0 in range(0, rows, NT):
                        n = min(NT, rows - nt0)
                        et = pre_et.tile([D + 1, NT], dt, tag="et")
                        nc.sync.dma_start(
                            et[:, :n], embT_t[:, base + nt0: base + nt0 + n]
                        )
                        ab_sb = pre_ab.tile([P, NT // P * 2 * D], dt, tag="absb")
                        assert n % (4 * P) == 0
                        for k4 in range(n // (4 * P)):
                            # 4 matmuls fill one PSUM bank -> one wide copy
                            ab_ps = ps_ab.tile([P, 512], mybir.dt.float32, space="PSUM")
                            for m in range(4):
                                k = 4 * k4 + m
                                nc.tensor.matmul(
                                    ab_ps[:, m * 2 * D:(m + 1) * 2 * D],
                                    lhsT=et[:, k * P:(k + 1) * P],
                                    rhs=w1cat_sb[:],
                                    start=True, stop=True,
                                )
                            nc.scalar.activation(
                                ab_sb[:, k4 * 512:(k4 + 1) * 512],
                                ab_ps[:],
                                mybir.ActivationFunctionType.Copy,
                            )
                        if BF16:
                            # host permuted emb columns so matmul k covers
                            # device nodes {nt0 + (k//M)*(M*128) + M*p + k%M}:
                            # rows M*p..M*p+M-1 of each group sit on partition
                            # p -> M*256B contiguous write descriptors.
                            M = PAIR_M
                            assert n % (M * P) == 0
                            dst = tabs[c][nt0:nt0 + n, :].rearrange(
                                "(g p m) f -> p g m f", p=P, m=M
                            )
                            src_ap = ab_sb[:, :n // P * 2 * D].rearrange(
                                "p (g m f) -> p g m f", m=M, f=2 * D
                            )
                        else:
                            # AB rows nt0+k*128+p , feat f  <- ab_sb[p, k*128+f]
                            dst = tabs[c][nt0:nt0 + n, :].rearrange(
                                "(k p) f -> p k f", p=P
                            )
                            src_ap = ab_sb[:, :n // P * 2 * D].rearrange(
                                "p (k f) -> p k f", f=2 * D
                            )
                        nc.sync.dma_start(dst, src_ap)

                # bucket schedule: emit bucket (cs,cd) after chunks cs, cd
                emitted_chunks = set()
                idx_off = 0   # in int16 columns (tot_idx//16 total)
                out_off = 0   # in slots

                gq = [0]

                def emit_bucket(b, tabs):
                    nonlocal idx_off, out_off
                    cs, cd = b // N_CHUNKS, b % N_CHUNKS
                    slots = s_b[b]
                    done = 0
                    while done < slots:
                        g = min(MAXG // P, slots - done)      # slots this gather
                        n_idx = g * P
                        isrc = idxp.tile([P, MAXG // 16], mybir.dt.int16, tag="isrc")
                        idst = idxp.tile([P, MAXG // 16], mybir.dt.int16, tag="idst")
                        nc.sync.dma_start(
                            isrc[:, :n_idx // 16],
                            isrc_t[:, idx_off: idx_off + n_idx // 16],
                        )
                        nc.sync.dma_start(
                            idst[:, :n_idx // 16],
                            idst_t[:, idx_off: idx_off + n_idx // 16],
                        )
                        if BF16:
                            # full 256B rows; use A half of src, B half of dst
                            ga = gap.tile([P, MAXG // P, 2 * D], dt, tag="ga")
                            gb = gap.tile([P, MAXG // P, 2 * D], dt, tag="gb")
                            ga_h = ga[:, :g, 0:D]
                            gb_h = gb[:, :g, D:2 * D]
                            gath = [
                                (ga, tabs[cs][:, :], isrc),
                                (gb, tabs[cd][:, :], idst),
                            ]
                            esz, estep = 2 * D, 2 * D
                        else:
                            ga = gap.tile([P, MAXG // P, D], mybir.dt.float32, tag="ga")
                            gb = gap.tile([P, MAXG // P, D], mybir.dt.float32, tag="gb")
                            ga_h = ga[:, :g, :]
                            gb_h = gb[:, :g, :]
                            gath = [
                                (ga, tabs[cs][:, 0:D], isrc),
                                (gb, tabs[cd][:, D:2 * D], idst),
                            ]
                            esz, estep = D, 2 * D
                        if mode not in ("B_dve",):
                            ns = SPLIT_GATHER if (g * P) % (SPLIT_GATHER * P) == 0 else 1
                            npart = n_idx // ns
                            for gt, src_ap, it in gath:
                                for si in range(ns):
                                    nc.gpsimd.dma_gather(
                                        gt[:, si * (g // ns):(si + 1) * (g // ns), :],
                                        src_ap,
                                        it[:, si * npart // 16:(si + 1) * npart // 16],
                                        npart, npart, esz, elem_step=estep,
                                        single_packet=SINGLE_PACKET, queue_num=gq[0],
                                    )
                                    gq[0] = (gq[0] + 1) % queues
                        # h = relu(ga + gb); logit = sum(h*w2) + b2
                        if mode in ("B_gather", "B_gp"):
                            idx_off += n_idx // 16
                            out_off += g
                            done += g
                            continue
                        lg = lgall[:, out_off:out_off + g]
                        if BF16:
                            # h in a separate CONTIGUOUS tile (unit-stride DVE,
                            # ga/gb release right after the add), then ONE
                            # fused DVE op does relu AND the +-1 sign multiply
                            # (|w2| pre-folded into the table), then a single
                            # full-width reduce straight into the persistent
                            # output buffer (one big DMA per iteration).
                            h = hp.tile([P, MAXG // P, D], dt, tag="h")
                            ga_h = h[:, :g, :]
                            nc.vector.tensor_add(ga_h, ga[:, :g, 0:D], gb_h)
                            nc.vector.scalar_tensor_tensor(
                                ga_h, ga_h, 0.0,
                                w2rep_sb[:, :g * D].rearrange("p (s d) -> p s d", d=D),
                                op0=mybir.AluOpType.max,
                                op1=mybir.AluOpType.mult,
                            )
                            nc.vector.tensor_reduce(
                                lg, ga_h,
                                axis=mybir.AxisListType.X,
                                op=mybir.AluOpType.add,
                            )
                        else:
                            nc.vector.tensor_add(ga_h, ga_h, gb_h)
                            nc.scalar.activation(
                                ga_h, ga_h, mybir.ActivationFunctionType.Relu,
                            )
                            nc.vector.tensor_mul(
                                ga_h, ga_h,
                                w2rep_sb[:, :g * D].rearrange("p (s d) -> p s d", d=D),
                            )
                            nc.vector.tensor_reduce(
                                lg, ga_h,
                                axis=mybir.AxisListType.X, op=mybir.AluOpType.add,
                            )
                        if not b2_zero:
                            nc.vector.tensor_scalar_add(lg, lg, b2b_sb[:, :1])
                        idx_off += n_idx // 16
                        out_off += g
                        done += g

                # schedule: chunks in order; after chunk c, all buckets whose
                # max(cs, cd) == c
                def emit_all(tabs):
                    nonlocal idx_off, out_off
                    idx_off = 0
                    out_off = 0
                    for c in range(N_CHUNKS):
                        precompute_chunk(c, tabs)
                        emitted_chunks.add(c)
                        for b in range(N_CHUNKS * N_CHUNKS):
                            cs, cd = b // N_CHUNKS, b % N_CHUNKS
                            if max(cs, cd) == c and s_b[b] > 0:
                                emit_bucket(b, tabs)
                    if mode != "B_gather":
                        nc.sync.dma_start(out_t[:], lgall[:])

                def emit_buckets_only(tabs):
                    nonlocal idx_off, out_off
                    idx_off = 0
                    out_off = 0
                    for c in range(N_CHUNKS):
                        for b in range(N_CHUNKS * N_CHUNKS):
                            cs, cd = b // N_CHUNKS, b % N_CHUNKS
                            if max(cs, cd) == c and s_b[b] > 0:
                                emit_bucket(b, tabs)
                    if mode not in ("B_gather", "B_gp"):
                        nc.sync.dma_start(out_t[:], lgall[:])

                if loop_n is None:
                    emit_all(ab_sets[0])
                elif mode in ("full", "B_gp"):
                    with tc.For_i(0, loop_n, 1):
                        for abs_u in ab_sets:
                            emit_all(abs_u)
                else:
                    for c in range(N_CHUNKS):
                        precompute_chunk(c, ab_sets[0])
                    with tc.For_i(0, loop_n, 1):
                        for abs_u in ab_sets:
                            emit_buckets_only(abs_u)

    nc.compile()
    names = dict(
        embT=embT_t.name, w1cat=w1cat_t.name, w2rep=w2rep_t.name,
        b2b=b2b_t.name, isrc=isrc_t.name, idst=idst_t.name, out=out_t.name,
    )
    return nc, names, tot_slots


def prepare(nodes_emb, src, dst, W1, b1, W2, b2):
    """Host prep: bucket sort, index packing, input arrays. Returns a dict."""
    nodes_emb = np.ascontiguousarray(np.asarray(nodes_emb, dtype=np.float32))
    src = np.asarray(src).astype(np.int64)
    dst = np.asarray(dst).astype(np.int64)
    W1 = np.asarray(W1, dtype=np.float32)
    b1 = np.asarray(b1, dtype=np.float32).reshape(-1)
    W2 = np.asarray(W2, dtype=np.float32)
    b2 = np.asarray(b2, dtype=np.float32).reshape(-1)
    E = src.shape[0]
    N_CHUNKS, R_PAD = _derived()

    # ---- host prep -------------------------------------------------------
    # embT65 [65, R_PAD]: emb^T padded with zeros, plus a ones row (bias)
    embT = np.zeros((D + 1, R_PAD), dtype=np.float32)
    embT[:D, :N_NODES] = nodes_emb.T
    embT[D, :] = 1.0
    # w1cat65 [65, 128]: cols 0:64 -> A-half (W1[:64] with b1), 64:128 -> B-half
    w1cat = np.zeros((D + 1, 2 * D), dtype=np.float32)
    w1cat[:D, :D] = W1[:D]
    w1cat[:D, D:] = W1[D:]
    w1cat[D, :D] = b1            # bias folded into A-half
    w2rep = np.tile(W2.reshape(1, D), (P, 1)).astype(np.float32)
    b2b = np.full((P, 1), b2[0], dtype=np.float32)
    npos = D
    if BF16:
        import ml_dtypes
        # fold |w2| into the table columns; permute features so +sign w2
        # features come first (cols [0:npos]) and -sign after.
        w2v = W2.reshape(D)
        fperm = np.argsort(w2v < 0, kind="stable")     # positives first
        npos = int((w2v >= 0).sum())
        scale = np.abs(w2v[fperm])                     # >= 0
        w1cat = w1cat[:, np.concatenate([fperm, D + fperm])] * np.concatenate([scale, scale])
        # device matmul k covers table rows (k//M)*(M*128) + M*p + (k%M);
        # permute emb columns so table row r holds node r's data.
        M = PAIR_M
        j = np.arange(R_PAD)
        k, p_ = j // P, j % P
        perm = (k // M) * (M * P) + M * p_ + (k % M)
        embT = embT[:, perm].astype(ml_dtypes.bfloat16)
        w1cat = w1cat.astype(ml_dtypes.bfloat16)
        sgn = np.where(w2v[fperm] >= 0, 1.0, -1.0).astype(np.float32)
        w2rep = np.tile(sgn.reshape(1, D), (P, 1)).astype(ml_dtypes.bfloat16)

    # ---- bucket sort + deal over cores ----------------------------------
    cs = src // CHUNK
    cd = dst // CHUNK
    bucket = (cs * N_CHUNKS + cd).astype(np.int64)
    if SORT_SRC:
        # ascending src rows inside each bucket: gather descriptors hit
        # increasing HBM addresses (row-buffer friendly)
        order = np.lexsort((src, bucket))
    else:
        order = np.argsort(bucket, kind="stable")      # edge ids, bucket-major
    bcounts = np.bincount(bucket, minlength=N_CHUNKS * N_CHUNKS)
    # per-core-per-bucket count (round-robin deal), padded to 128
    m_bc = -(-bcounts // N_CORES)                      # ceil
    s_b = [int(_round_up(m, P) // P) if m > 0 else 0 for m in m_bc]
    tot_slots = int(sum(s_b))
    tot_idx = tot_slots * P

    src_s = (src[order] % CHUNK).astype(np.int16)
    dst_s = (dst[order] % CHUNK).astype(np.int16)

    # host index bookkeeping: for each sorted position, compute its
    # (core, flat device stream index)
    core_of = np.empty(E, dtype=np.int64)
    stream_of = np.empty(E, dtype=np.int64)
    bstart = np.concatenate([[0], np.cumsum(bcounts)])
    # device consumes buckets in emit order: bucket (cs, cd) is emitted after
    # AB chunks cs and cd, i.e. grouped by max(cs, cd)
    emit_order = [
        b for c in range(N_CHUNKS)
        for b in range(N_CHUNKS * N_CHUNKS)
        if max(b // N_CHUNKS, b % N_CHUNKS) == c
    ]
    slot_off = np.zeros(N_CHUNKS * N_CHUNKS, dtype=np.int64)
    acc = 0
    for b in emit_order:
        slot_off[b] = acc
        acc += s_b[b]
    for b in range(N_CHUNKS * N_CHUNKS):
        nb = bcounts[b]
        if nb == 0:
            continue
        pos = np.arange(nb)
        core_of[bstart[b]: bstart[b + 1]] = pos % N_CORES
        stream_of[bstart[b]: bstart[b + 1]] = slot_off[b] * P + pos // N_CORES

    isrc_all = np.zeros((N_CORES, tot_idx), dtype=np.int16)
    idst_all = np.zeros((N_CORES, tot_idx), dtype=np.int16)
    for c in range(N_CORES):
        m = core_of == c
        isrc_all[c, stream_of[m]] = src_s[m]
        idst_all[c, stream_of[m]] = dst_s[m]

    def wrap16(a):
        # stream index i -> [i % 16, i // 16], replicated to 128 partitions
        w = a.reshape(-1, 16).T                        # [16, tot/16]
        return np.tile(w, (8, 1)).copy()

    return dict(
        E=E, s_b=s_b, npos=npos, b2_zero=bool(b2[0] == 0.0),
        core_of=core_of, stream_of=stream_of,
        order=order, embT=embT, w1cat=w1cat, w2rep=w2rep, b2b=b2b,
        isrc=[wrap16(isrc_all[c]) for c in range(N_CORES)],
        idst=[wrap16(idst_all[c]) for c in range(N_CORES)],
    )


def make_in_maps(prep, names):
    return [
        {
            names["embT"]: prep["embT"],
            names["w1cat"]: prep["w1cat"],
            names["w2rep"]: prep["w2rep"],
            names["b2b"]: prep["b2b"],
            names["isrc"]: prep["isrc"][c],
            names["idst"]: prep["idst"][c],
        }
        for c in range(N_CORES)
    ]


def run_prep(prep, loop_n=None, unroll2=False):
    """Build the program (optionally with a repeat loop) and run it once."""
    from concourse.bass_utils import run_bass_kernel_spmd

    nc, names, tot_slots = _build_program(
        prep["s_b"], npos=prep.get("npos", D), loop_n=loop_n, queues=QUEUES,
        unroll2=unroll2, b2_zero=prep.get("b2_zero", False),
    )
    in_maps = make_in_maps(prep, names)
    res = run_bass_kernel_spmd(
        nc, in_maps, core_ids=list(range(N_CORES)), trace=False,
    )
    global _last_results, _last_ctx, _last_names, _last_s_b
    _last_results = res
    _last_ctx = (nc, in_maps)
    _last_names = names
    _last_s_b = prep["s_b"]
    return res, names


def unscramble(prep, res, names):
    # device out [128, tot_slots]: stream index i -> out[i % 128, i // 128]
    E = prep["E"]
    core_of, stream_of, order = prep["core_of"], prep["stream_of"], prep["order"]
    logits_sorted = np.empty(E, dtype=np.float32)
    for c in range(N_CORES):
        o = res.results[c][names["out"]]               # [128, tot_slots]
        m = core_of == c
        si = stream_of[m]
        logits_sorted[np.flatnonzero(m)] = o[si % P, si // P]
    out = np.empty(E, dtype=np.float32)
    out[order] = logits_sorted
    return out.reshape(E, 1)


def kernel(nodes_emb, src, dst, W1, b1, W2, b2):
    prep = prepare(nodes_emb, src, dst, W1, b1, W2, b2)
    res, names = run_prep(prep, loop_n=None)
    return unscramble(prep, res, names)


def measure_hw(prep, r1=8, r2=64, n_iters=10, unroll2=False):
    """Differential HW timing: the program body repeated r inside one NEFF
    dispatch; per-execution time = (wall(r2) - wall(r1)) / (execs2 - execs1).

    The axon/PJRT dispatch overhead (~80 ms, validated with a trivial
    kernel) cancels in the difference.  With unroll2, each loop iteration
    holds TWO complete kernel executions on alternating DRAM table sets
    (removes the artificial write-after-read serialization at the loop
    back-edge that a single-shot run does not have).  Also verifies the
    loop programs produce the same outputs as the single-shot program.
    """
    mult = 2 if unroll2 else 1
    walls = {}
    outs = {}
    for r in (r1, r2):
        res, names = run_prep(prep, loop_n=r, unroll2=unroll2)
        outs[r] = unscramble(prep, res, names)
        ts = bench(n_iters=n_iters)
        walls[r] = min(ts)
        print(f"  loop_n={r} (x{mult}): wall min {walls[r]*1e3:.2f} ms "
              f"(iters: {[f'{t*1e3:.1f}' for t in ts]})")
    per_iter = (walls[r2] - walls[r1]) / ((r2 - r1) * mult)
    return per_iter, outs[r1], outs[r2]


def bench(n_iters=16, n_warmup=3):
    """Re-execute the last-compiled SPMD program on device-resident inputs.

    Returns list of per-iteration wall seconds (device exec + dispatch).
    """
    import time
    import jax
    import numpy as np
    from jax.sharding import Mesh, PartitionSpec
    from jax.experimental.shard_map import shard_map
    import concourse.mybir as mybir
    from concourse import bass2jax

    nc, in_maps = _last_ctx
    n_cores = len(in_maps)
    partition_name = nc.partition_id_tensor.name if nc.partition_id_tensor else None

    in_names, out_names, out_avals, zero_outs = [], [], [], []
    for alloc in nc.m.functions[0].allocations:
        if not isinstance(alloc, mybir.MemoryLocationSet):
            continue
        name = alloc.memorylocations[0].name
        if alloc.kind == "ExternalInput":
            if name != partition_name:
                in_names.append(name)
        elif alloc.kind == "ExternalOutput":
            shape = tuple(alloc.tensor_shape)
            dtype = mybir.dt.np(alloc.dtype)
            out_names.append(name)
            out_avals.append(jax.core.ShapedArray(shape, dtype))
            zero_outs.append(np.zeros(shape, dtype))
    n_params = len(in_names)
    n_outs = len(out_avals)
    in_names_all = in_names + out_names
    if partition_name is not None:
        in_names_all = in_names_all + [partition_name]

    def _body(*args):
        operands = list(args)
        if partition_name is not None:
            operands.append(bass2jax.partition_id_tensor())
        outs = bass2jax._bass_exec_p.bind(
            *operands,
            out_avals=tuple(out_avals),
            in_names=tuple(in_names_all),
            out_names=tuple(out_names),
            lowering_input_output_aliases=(),
            sim_require_finite=True,
            sim_require_nnan=True,
            nc=nc,
        )
        return tuple(outs)

    devices = jax.devices()[:n_cores]
    mesh = Mesh(np.asarray(devices), ("core",))
    in_specs = (PartitionSpec("core"),) * (n_params + n_outs)
    out_specs = (PartitionSpec("core"),) * n_outs
    donate = tuple(range(n_params, n_params + n_outs))
    sharded = jax.jit(
        shard_map(_body, mesh=mesh, in_specs=in_specs, out_specs=out_specs,
                  check_rep=False),
        donate_argnums=donate, keep_unused=True,
    )
    sharding = jax.sharding.NamedSharding(mesh, PartitionSpec("core"))
    concat_in = [
        jax.device_put(
            np.concatenate([np.asarray(in_maps[c][name]) for c in range(n_cores)], axis=0),
            sharding,
        )
        for name in in_names
    ]
    jax.block_until_ready(concat_in)
    n_total = n_warmup + n_iters
    zero_sets = [
        [
            jax.device_put(
                np.zeros((n_cores * z.shape[0], *z.shape[1:]), z.dtype), sharding
            )
            for z in zero_outs
        ]
        for _ in range(n_total)
    ]
    jax.block_until_ready(zero_sets)

    times = []
    for i in range(n_total):
        t0 = time.perf_counter()
        out = sharded(*concat_in, *zero_sets[i])
        jax.block_until_ready(out)
        times.append(time.perf_counter() - t0)
        del out
    return times[n_warmup:]

